# revision 41
# baseline (speedup 1.0000x reference)
"""GCN discriminator kernel for Trainium2 (8 NeuronCores, SPMD).

Math (matching the reference):
  deg[n]  = sum_{e: dst=n} w_e + 1
  dinv    = 1/sqrt(deg)
  norm_e  = dinv[src]*w_e*dinv[dst];  self-loop n: dinv[n]^2
  agg     = sum over incoming edges of norm_e * x[src]         [N, 128]
  h       = leaky_relu(agg @ W1 + b1)                          [N, 256]
  pooled  = segment_mean(h, batch)                             [64, 256]
  z       = leaky_relu(concat(pooled, emb[cls]) @ W2 + b2)
  out     = z @ W3 + b3                                        [64, 1]

Sharding: graph-aligned.  batch is sorted, so graphs occupy contiguous node
ranges; each core owns 8 whole graphs (and the nodes/edge-destinations that
fall in them).  Pooling is therefore core-local — no collective at all; each
core runs the tiny MLP on its own 8 graphs and the host concatenates the
8x[8,1] results.

The irregular gather x[src] runs as HBM dma_gather (SWDGE descriptors)
spread over all 4 SWDGE queues so descriptor emission on the Q7 core-pairs
overlaps the SDMA drains.  Indices are signed-int16 offsets from row XOFF so
one gather covers all 50k source rows.  Self-loop contributions read
contiguous x rows (dense DMA) and are folded in on the Vector engine during
the PSUM evacuation.  The edge segment-sum runs on the PE as one-hot
(weight-scaled) matmuls; pooling is a count-scaled one-hot matmul
accumulated in PSUM.
"""

import numpy as np
import ml_dtypes
from collections import defaultdict

# ----------------------------------------------------------------- config
CFG = dict(
    N=50000, F=128, HID=256, G=64, NCLS=10,
    NCORES=8,
    XOFF=25000,           # gather base row (signed int16 offsets)
    ST_D=128,             # dsts per supertile (= matmul M)
    K=128,                # edge slots per chunk (matmul contraction)
    GRAN=2,               # supertiles per gather granule
    TAPER=4,              # trailing supertiles run as single-st granules
    NEG=0.2,
    NQ=4,                 # SWDGE queues used round-robin for gathers
    SPLIT=2,              # gathers per granule (parallel Q7 emission)
    NO_GATHER=0,          # debug: memset instead of dma_gather
    GDT="bf16",           # gather dtype: "f32" | "bf16"  (x rows + S weights)
    MMDT="bf16",          # downstream matmul dtype
)


def _np_dt(s):
    return {"f32": np.float32, "bf16": ml_dtypes.bfloat16}[s]


# ================================================================= host prep
class Prep:
    pass


def host_prep(inputs, cfg):
    """Integer/layout preprocessing + normalization weights.

    Returns per-core in_maps plus the static (core-independent) program
    structure.
    """
    N, F, G = cfg["N"], cfg["F"], cfg["G"]
    NC, XOFF = cfg["NCORES"], cfg["XOFF"]
    ST_D, K, GRAN = cfg["ST_D"], cfg["K"], cfg["GRAN"]
    GL = G // NC                                   # graphs per core

    x = np.asarray(inputs["x"], np.float32)
    ei = np.asarray(inputs["edge_index"], np.int64)
    ew = np.asarray(inputs["edge_weight"], np.float32)
    batch = np.asarray(inputs["batch"], np.int64)
    cls = np.asarray(inputs["class_labels"], np.int64)
    W1 = np.asarray(inputs["W1"], np.float32)
    b1 = np.asarray(inputs["b1"], np.float32)
    emb = np.asarray(inputs["emb"], np.float32)
    W2 = np.asarray(inputs["W2"], np.float32)
    b2 = np.asarray(inputs["b2"], np.float32)
    W3 = np.asarray(inputs["W3"], np.float32)
    b3 = np.asarray(inputs["b3"], np.float32)

    HID = W1.shape[1]
    EH = emb.shape[1]

    # --- normalization weights (scalar preprocessing, O(E)) -------------
    row = ei[0]
    col = ei[1]
    deg = np.zeros(N, np.float64)
    np.add.at(deg, col, ew.astype(np.float64))
    deg += 1.0
    dinv = 1.0 / np.sqrt(deg)
    a_w = (dinv[row] * ew.astype(np.float64) * dinv[col]).astype(np.float32)
    a_src = row
    a_dst = col
    selfw = (dinv * dinv).astype(np.float32)       # self-loop weights [N]

    # --- balanced graph->core assignment (LPT bin packing, 8 per core) ----
    gcnt = np.bincount(batch, minlength=G)           # nodes per graph
    gcum = np.concatenate([[0], np.cumsum(gcnt)])    # graph start offsets
    load = np.zeros(NC, np.int64)
    nfill = np.zeros(NC, np.int64)
    asg = [[] for _ in range(NC)]                    # graphs per core
    for g in np.argsort(-gcnt, kind="stable"):
        c = min((c for c in range(NC) if nfill[c] < GL),
                key=lambda c: load[c])
        asg[c].append(int(g))
        load[c] += gcnt[g]
        nfill[c] += 1
    nlists = []                                      # per-core node ids
    core_of_node = np.empty(N, np.int64)
    loc_of_node = np.empty(N, np.int64)
    for c in range(NC):
        nl = np.concatenate(
            [np.arange(gcum[g], gcum[g + 1]) for g in asg[c]])
        nlists.append(nl)
        core_of_node[nl] = c
        loc_of_node[nl] = np.arange(len(nl))
    Dmax = int(load.max())
    NST = -(-Dmax // ST_D)      # supertiles per core

    # --- bucket edges into (core, st) ------------------------------------
    core_of = core_of_node[a_dst]
    dst_loc = loc_of_node[a_dst]
    st_of = dst_loc // ST_D
    j_of = dst_loc % ST_D
    key = core_of * NST + st_of

    nbuckets = NC * NST
    cnt = np.bincount(key, minlength=nbuckets).reshape(NC, NST)
    starts = np.zeros(nbuckets + 1, np.int64)
    np.cumsum(cnt.reshape(-1), out=starts[1:])

    # Per-core x-row permutation: order nodes by first-use bucket so each
    # bucket's "home" sources occupy consecutive rows — their 256B gather
    # reads become sequential (HBM row-buffer hits).  Buckets are then
    # sorted by permuted id, making home reads ascending runs.
    perms = []
    seg_src = []   # per-core permuted signed src offsets, bucket-sorted
    seg_j = []
    seg_w = []
    for c in range(NC):
        mask = core_of == c
        st_c = st_of[mask]
        src_c = a_src[mask]
        j_c = j_of[mask]
        w_c = a_w[mask]
        o0 = np.lexsort((src_c, st_c))
        seq = src_c[o0]
        uniq, first_idx = np.unique(seq, return_index=True)
        order_nodes = uniq[np.argsort(first_idx)]
        perm = np.full(N, -1, np.int64)
        perm[order_nodes] = np.arange(len(order_nodes))
        rest = np.nonzero(perm < 0)[0]
        perm[rest] = len(order_nodes) + np.arange(len(rest))
        p_src = perm[src_c]
        o1 = np.lexsort((p_src, st_c))
        perms.append(perm)
        seg_src.append((p_src[o1] - XOFF).astype(np.int32))
        seg_j.append(j_c[o1].astype(np.int32))
        seg_w.append(w_c[o1])

    # static chunk counts per st: max over cores, >= 1; the +1 guarantees
    # at least one padding slot per bucket so every gather's trailing slot
    # is the non-negative padding index (the ucode drops trailing
    # negative-idx runs)
    CH = np.maximum((-(-(cnt + 1) // K)).max(axis=0), 1)   # [NST]

    # --- static program structure ---------------------------------------
    TAPER = cfg.get("TAPER", 0)
    grans = []
    st = 0
    while st < NST:
        n = 1 if st >= NST - TAPER else min(GRAN, NST - st)
        grans.append((st, n))
        st += n

    SELF = -1                                      # self-loop chunk marker
    gmeta = []
    mm_total = 0
    col_total = 0
    for (st0, nst) in grans:
        sts = list(range(st0, st0 + nst))
        nch = int(sum(CH[st] for st in sts))
        cimap = {}
        mms = []
        ci = 0
        for st in sts:
            for k in range(int(CH[st])):
                cimap[(st, k)] = ci
                mms.append((st, k))
                ci += 1
            mms.append((st, SELF))                 # diag(selfw) @ xself[st]
        gmeta.append(dict(st0=st0, nst=nst, nch=nch,
                          cimap=cimap, mms=mms,
                          mm_off=mm_total, col_off=col_total))
        mm_total += len(mms)
        col_total += nch * (K // 16)
    NMM = mm_total
    NCOLS = col_total

    static = dict(cfg=cfg, NST=NST, CH=CH, grans=grans, gmeta=gmeta,
                  NMM=NMM, NCOLS=NCOLS, HID=HID, EH=EH, GL=GL, asg=asg)

    # --- per-core tensors ------------------------------------------------
    gdt = _np_dt(cfg["GDT"])
    mmdt = _np_dt(cfg["MMDT"])

    counts = np.maximum(gcnt, 1).astype(np.float64)

    w2blk = np.zeros((128, 6 * 128), np.float32)
    for kk in range(3):
        for jj in range(2):
            w2blk[:, (kk * 2 + jj) * 128:(kk * 2 + jj + 1) * 128] = \
                W2[kk * 128:(kk + 1) * 128, jj * 128:(jj + 1) * 128]
    w3m = np.zeros((128, 2), np.float32)
    w3m[:, 0] = W3[0:128, 0]
    w3m[:, 1] = W3[128:256, 0]

    xg = np.ascontiguousarray(x).astype(gdt)         # gather/dense source

    in_maps = []
    for c in range(NC):
        cstart = starts[c * NST]
        srcl_s = seg_src[c]
        j_s = seg_j[c]
        w_s = seg_w[c]
        perm = perms[c]
        # permuted per-core x copy: new row perm[i] holds x[i]
        xgc = np.empty_like(xg)
        xgc[perm] = xg
        gidx = np.zeros((NCOLS, 16), np.int16)
        smat = np.zeros((128, NMM * ST_D), gdt)
        for gi, gm in enumerate(gmeta):
            nch = gm["nch"]
            slots_src = np.zeros((nch * K,), np.int32)
            for (st_k, ci) in gm["cimap"].items():
                st, k = st_k
                b = c * NST + st
                s0, s1 = starts[b] - cstart, starts[b + 1] - cstart
                seg = srcl_s[s0:s1]
                part = seg[k * K:(k + 1) * K]
                slots_src[ci * K: ci * K + len(part)] = part
            # gather ucode drops a trailing negative-idx run; gathers are
            # bucket-aligned and padding slots are 0 (>= 0), so this only
            # triggers if a bucket's final chunk is exactly full of
            # all-sub-XOFF sources — vanishingly unlikely with
            # ascending-src bucket order.
            ce = 0
            for st in range(gm["st0"], gm["st0"] + gm["nst"]):
                ce += int(CH[st])
                assert slots_src[ce * K - 1] >= 0, "trailing negative idx"
            c0 = gm["col_off"]
            gidx[c0:c0 + nch * (K // 16), :] = slots_src.reshape(-1, 16)
            # S matrices
            for mi, (st, k) in enumerate(gm["mms"]):
                mm = gm["mm_off"] + mi
                if k < 0:
                    # self-loop diag chunk: agg += diag(selfw) @ xself[st]
                    nl = nlists[c]
                    d0 = st * ST_D
                    nvalid = min(ST_D, max(0, len(nl) - d0))
                    rows = np.arange(nvalid)
                    smat[rows, mm * ST_D + rows] = \
                        selfw[nl[d0:d0 + nvalid]].astype(gdt)
                    continue
                b = c * NST + st
                s0, s1 = starts[b] - cstart, starts[b + 1] - cstart
                jj = j_s[s0:s1][k * K:(k + 1) * K]
                ww = w_s[s0:s1][k * K:(k + 1) * K]
                rows = np.arange(len(jj))
                smat[rows, mm * ST_D + jj] = ww.astype(gdt)
        gidx2 = np.tile(gidx.T, (8, 1))              # [128, NCOLS]

        nl = nlists[c]
        gslot = {g: j for j, g in enumerate(asg[c])}

        # pooling matrix [128, NST*GL], scaled by 1/count (mean pooling)
        pmat = np.zeros((128, NST * GL), np.float64)
        for stn in range(NST):
            d0 = stn * ST_D
            nv = min(ST_D, max(0, len(nl) - d0))
            for p in range(nv):
                g = batch[nl[d0 + p]]
                pmat[p, stn * GL + gslot[g]] = 1.0 / counts[g]

        # self-loop x rows: the core's local nodes in local order, padded
        xself = np.zeros((NST * ST_D, F), gdt)
        xself[:len(nl)] = xg[nl]

        # class embeddings for this core's graphs: clt [NCLS, GL]
        clt = np.zeros((cfg["NCLS"], GL), mmdt)
        clt[cls[asg[c]], np.arange(GL)] = 1.0

        m = dict(
            xg=xgc, xself=xself,
            gidx=np.ascontiguousarray(gidx2),
            smat=smat,
            pmat=pmat.astype(mmdt),
            w1=W1.astype(mmdt),
            w2blk=w2blk.astype(mmdt),
            w3=w3m.astype(mmdt),
            b1=b1.reshape(1, HID).astype(mmdt),
            b2=b2.reshape(1, HID).astype(mmdt),
            b3=b3.reshape(1, 1).astype(mmdt),
            embh=emb.astype(mmdt),
            clt=clt,
        )
        in_maps.append(m)

    prep = Prep()
    prep.static = static
    prep.in_maps = in_maps
    return prep


# ================================================================= builder
def build(static):
    import concourse.bass as bass
    from concourse import bacc, tile
    import concourse.mybir as mybir

    cfg = static["cfg"]
    N, F = cfg["N"], cfg["F"]
    XOFF = cfg["XOFF"]
    ST_D, K = cfg["ST_D"], cfg["K"]
    NST, CH, gmeta = static["NST"], static["CH"], static["gmeta"]
    NMM, NCOLS = static["NMM"], static["NCOLS"]
    HID, EH, GL = static["HID"], static["EH"], static["GL"]
    NCLS = cfg["NCLS"]
    NEG = cfg["NEG"]
    NQ = cfg["NQ"]

    gdt = {"f32": mybir.dt.float32, "bf16": mybir.dt.bfloat16}[cfg["GDT"]]
    mmdt = {"f32": mybir.dt.float32, "bf16": mybir.dt.bfloat16}[cfg["MMDT"]]
    f32 = mybir.dt.float32
    AF = mybir.ActivationFunctionType

    nc = bacc.Bacc(None, target_bir_lowering=False, debug=True,
                   num_swdge_queues=NQ)

    xg_d = nc.declare_dram_parameter("xg", [N, F], gdt, isOutput=False)
    xself_d = nc.declare_dram_parameter("xself", [NST * ST_D, F], gdt, isOutput=False)
    gidx_d = nc.declare_dram_parameter("gidx", [128, NCOLS], mybir.dt.int16, isOutput=False)
    smat_d = nc.declare_dram_parameter("smat", [128, NMM * ST_D], gdt, isOutput=False)
    pmat_d = nc.declare_dram_parameter("pmat", [128, NST * GL], mmdt, isOutput=False)
    w1_d = nc.declare_dram_parameter("w1", [F, HID], mmdt, isOutput=False)
    w2_d = nc.declare_dram_parameter("w2blk", [128, 6 * 128], mmdt, isOutput=False)
    w3_d = nc.declare_dram_parameter("w3", [128, 2], mmdt, isOutput=False)
    b1_d = nc.declare_dram_parameter("b1", [1, HID], mmdt, isOutput=False)
    b2_d = nc.declare_dram_parameter("b2", [1, HID], mmdt, isOutput=False)
    b3_d = nc.declare_dram_parameter("b3", [1, 1], mmdt, isOutput=False)
    emb_d = nc.declare_dram_parameter("embh", [NCLS, EH], mmdt, isOutput=False)
    clt_d = nc.declare_dram_parameter("clt", [NCLS, GL], mmdt, isOutput=False)
    out_d = nc.declare_dram_parameter("out", [1, GL], f32, isOutput=True)

    iden_np = np.eye(128, dtype=_np_dt(cfg["MMDT"]))
    iden_d = nc.inline_tensor(iden_np, name="iden")

    with tile.TileContext(nc) as tc:
        with (
            tc.tile_pool(name="const", bufs=1) as constp,
            tc.tile_pool(name="gix", bufs=4) as gixp,
            tc.tile_pool(name="gat", bufs=4) as gatp,
            tc.tile_pool(name="xsf", bufs=4) as xsfp,
            tc.tile_pool(name="smp", bufs=3) as smp,
            tc.tile_pool(name="work", bufs=3) as workp,
            tc.tile_pool(name="ps_agg", bufs=2, space="PSUM") as ps_agg,
            tc.tile_pool(name="ps_t", bufs=2, space="PSUM") as ps_t,
            tc.tile_pool(name="ps_h", bufs=2, space="PSUM") as ps_h,
            tc.tile_pool(name="ps_pool", bufs=1, space="PSUM") as ps_pool,
        ):
            # ---- persistent SBUF loads
            pmat_sb = constp.tile([128, NST * GL], mmdt)
            nc.sync.dma_start(out=pmat_sb[:, :], in_=pmat_d[:, :])
            w1_sb = constp.tile([F, HID], mmdt)
            nc.sync.dma_start(out=w1_sb[:, :], in_=w1_d[:, :])
            iden_sb = constp.tile([128, 128], mmdt)
            nc.sync.dma_start(out=iden_sb[:, :], in_=iden_d[:, :])
            b1_sb = constp.tile([1, HID], mmdt)
            nc.sync.dma_start(out=b1_sb[:, :], in_=b1_d[:, :])
            ones_sb = constp.tile([1, 128], mmdt)
            nc.vector.memset(ones_sb[:, :], 1.0)

            pooled_ps = ps_pool.tile([GL, HID], f32)

            # ---------------- main loop over granules
            qn = 0
            for gi, gm in enumerate(gmeta):
                st0, nst = gm["st0"], gm["nst"]
                nch = gm["nch"]
                c0 = gm["col_off"]
                gt = gatp.tile([128, nch, F], gdt, tag="gt")
                gx = gixp.tile([128, nch * (K // 16)], mybir.dt.int16, tag="gx")
                nc.scalar.dma_start(
                    out=gx[:, :],
                    in_=gidx_d[:, c0:c0 + nch * (K // 16)])
                if cfg.get("NO_GATHER"):
                    nc.vector.memset(gt[:, :, :], 0.125)
                else:
                    # one gather per supertile (bucket-aligned so the
                    # trailing-negative-idx drop can't hit real slots),
                    # spread over SWDGE queues for parallel Q7 emission
                    ca = 0
                    for st in range(st0, st0 + nst):
                        cb = ca + int(CH[st])
                        nc.gpsimd.dma_gather(
                            gt[:, ca:cb, :], xg_d[XOFF:, :],
                            gx[:, ca * (K // 16):cb * (K // 16)],
                            num_idxs=(cb - ca) * K,
                            num_idxs_reg=(cb - ca) * K,
                            elem_size=F, single_packet=bool(cfg.get("SP1", 0)),
                            queue_num=qn % NQ)
                        qn += 1
                        ca = cb

                # dense self-loop rows for these supertiles
                xsf = xsfp.tile([128, nst, F], gdt, tag="xsf")
                for sti in range(nst):
                    nc.scalar.dma_start(
                        out=xsf[:, sti, :],
                        in_=xself_d[(st0 + sti) * ST_D:(st0 + sti + 1) * ST_D, :])

                nmm_g = len(gm["mms"])
                sm_sb = smp.tile([128, nmm_g * ST_D], gdt, tag="sm")
                m0 = gm["mm_off"]
                nc.sync.dma_start(
                    out=sm_sb[:, :],
                    in_=smat_d[:, m0 * ST_D:(m0 + nmm_g) * ST_D])

                by_st = defaultdict(list)
                for mi, (st, k) in enumerate(gm["mms"]):
                    by_st[st].append((mi, k))

                for sti, st in enumerate(range(st0, st0 + nst)):
                    # S-matmuls with gathered rows STATIONARY: the PSUM
                    # tile accumulates agg^T = gt.T @ S directly in the
                    # [feature, dst] layout the W1 matmul wants as lhsT —
                    # no transpose step needed.
                    agg = ps_agg.tile([128, F], f32, tag="agg")
                    lst = by_st[st]
                    for i, (mi, k) in enumerate(lst):
                        if k < 0:
                            lhsT = xsf[:, sti, :]    # self-loop diag chunk
                        else:
                            lhsT = gt[:, gm["cimap"][(st, k)], :]
                        nc.tensor.matmul(
                            agg[:, :],
                            lhsT=lhsT,
                            rhs=sm_sb[:, mi * ST_D:(mi + 1) * ST_D],
                            start=(i == 0), stop=(i == len(lst) - 1))
                    aggT_sb = workp.tile([128, 128], mmdt, tag="aggT_sb")
                    nc.vector.tensor_copy(out=aggT_sb[:, :], in_=agg[:, :])
                    # W1 + b1
                    h_ps = ps_h.tile([128, HID], f32, tag="h")
                    nc.tensor.matmul(h_ps[:, :], lhsT=aggT_sb[:, :],
                                     rhs=w1_sb[:, :], start=True, stop=False)
                    nc.tensor.matmul(h_ps[:, :], lhsT=ones_sb[:, 0:128],
                                     rhs=b1_sb[:, :], start=False, stop=True)
                    # leaky relu -> sbuf
                    hr_sb = workp.tile([128, HID], f32, tag="hr_sb")
                    nc.scalar.activation(hr_sb[:, :], h_ps[:, :], AF.Relu,
                                         scale=1.0 - NEG)
                    h_sb = workp.tile([128, HID], mmdt, tag="h_sb")
                    nc.vector.scalar_tensor_tensor(
                        h_sb[:, :], in0=h_ps[:, :], scalar=NEG,
                        in1=hr_sb[:, :], op0=mybir.AluOpType.mult,
                        op1=mybir.AluOpType.add)
                    # mean-pool accumulate (pmat carries 1/count)
                    nc.tensor.matmul(
                        pooled_ps[:, :],
                        lhsT=pmat_sb[:, st * GL:(st + 1) * GL],
                        rhs=h_sb[:, :],
                        start=(st == 0), stop=(st == NST - 1),
                        skip_group_check=True)

            # ---------------- tail: core-local MLP on GL graphs
            pm_sb = workp.tile([GL, HID], mmdt, tag="pm")
            nc.vector.tensor_copy(out=pm_sb[:, :], in_=pooled_ps[:, :])

            # transpose pooled -> z^T rows [128, GL] halves
            zt = []
            for jj in range(HID // 128):
                tp = ps_t.tile([128, GL], mmdt, tag="aggT")
                nc.tensor.transpose(tp[:, :], pm_sb[:, jj * 128:(jj + 1) * 128],
                                    iden_sb[0:GL, 0:GL])
                t_sb = workp.tile([128, GL], mmdt, tag=f"zt{jj}")
                nc.scalar.copy(out=t_sb[:, :], in_=tp[:, :])
                zt.append(t_sb)
            # class-embedding^T [EH, GL]
            emb_sb = workp.tile([NCLS, EH], mmdt, tag="emb")
            nc.sync.dma_start(out=emb_sb[:, :], in_=emb_d[:, :])
            clt_sb = workp.tile([NCLS, GL], mmdt, tag="clt")
            nc.sync.dma_start(out=clt_sb[:, :], in_=clt_d[:, :])
            ce_ps = ps_t.tile([EH, GL], f32, tag="aggT")
            nc.tensor.matmul(ce_ps[:, :], lhsT=emb_sb[:, :], rhs=clt_sb[:, :],
                             start=True, stop=True)
            ce_sb = workp.tile([EH, GL], mmdt, tag="ce_sb")
            nc.scalar.copy(out=ce_sb[:, :], in_=ce_ps[:, :])
            zt.append(ce_sb)

            # W2: z2^T[128j] = sum_k W2blk[k,j].T @ zt[k]
            w2_sb = workp.tile([128, 6 * 128], mmdt, tag="w2")
            nc.sync.dma_start(out=w2_sb[:, :], in_=w2_d[:, :])
            b2_sb = workp.tile([1, HID], mmdt, tag="b2")
            nc.sync.dma_start(out=b2_sb[:, :], in_=b2_d[:, :])
            ones_g = workp.tile([1, GL], mmdt, tag="onesg")
            nc.vector.memset(ones_g[:, :], 1.0)
            nk = (HID + EH) // 128
            z2 = []
            for jj in range(2):
                zp = ps_h.tile([128, GL], f32, tag="h")
                for kk in range(nk):
                    nc.tensor.matmul(
                        zp[:, :],
                        lhsT=w2_sb[:, (kk * 2 + jj) * 128:(kk * 2 + jj + 1) * 128],
                        rhs=zt[kk][:, :], start=(kk == 0), stop=False)
                nc.tensor.matmul(zp[:, :], lhsT=b2_sb[:, jj * 128:(jj + 1) * 128],
                                 rhs=ones_g[:, :], start=False, stop=True)
                zr_sb = workp.tile([128, GL], f32, tag="zr_sb")
                nc.scalar.activation(zr_sb[:, :], zp[:, :], AF.Relu,
                                     scale=1.0 - NEG)
                z_sb = workp.tile([128, GL], mmdt, tag=f"z2sb{jj}")
                nc.vector.scalar_tensor_tensor(
                    z_sb[:, :], in0=zp[:, :], scalar=NEG, in1=zr_sb[:, :],
                    op0=mybir.AluOpType.mult, op1=mybir.AluOpType.add)
                z2.append(z_sb)

            w3_sb = workp.tile([128, 2], mmdt, tag="w3")
            nc.sync.dma_start(out=w3_sb[:, :], in_=w3_d[:, :])
            b3_sb = workp.tile([1, 1], mmdt, tag="b3")
            nc.sync.dma_start(out=b3_sb[:, :], in_=b3_d[:, :])
            op = ps_h.tile([1, GL], f32, tag="h")
            for jj in range(2):
                nc.tensor.matmul(op[:, :], lhsT=w3_sb[:, jj:jj + 1],
                                 rhs=z2[jj][:, :], start=(jj == 0), stop=False)
            nc.tensor.matmul(op[:, :], lhsT=b3_sb[:, :], rhs=ones_g[:, :],
                             start=False, stop=True)
            o_sb = workp.tile([1, GL], f32, tag="osb")
            nc.vector.tensor_copy(out=o_sb[:, :], in_=op[:, :])
            nc.sync.dma_start(out=out_d[:, :], in_=o_sb[:, :])

    return nc


# ================================================================= runner
def _run(inputs, cfg=None, trace=False):
    from concourse.bass_utils import run_bass_kernel_spmd
    cfg = dict(CFG if cfg is None else cfg)
    prep = host_prep(inputs, cfg)
    nc = build(prep.static)
    nc.finalize()
    res = run_bass_kernel_spmd(
        nc, prep.in_maps, core_ids=list(range(cfg["NCORES"])), trace=trace)
    G = cfg["G"]
    out = np.zeros((G, 1), np.float32)
    for c, r in enumerate(res.results):
        oc = np.asarray(r["out"], np.float32).reshape(-1)
        for j, g in enumerate(prep.static["asg"][c]):
            out[g, 0] = oc[j]
    return out, res


def kernel(**inputs):
    out, _ = _run(inputs)
    return out


# revision 42
# speedup vs baseline: 1.0966x; 1.0966x over previous
"""GCN discriminator kernel for Trainium2 (8 NeuronCores, SPMD).

Math (matching the reference):
  deg[n]  = sum_{e: dst=n} w_e + 1
  dinv    = 1/sqrt(deg)
  norm_e  = dinv[src]*w_e*dinv[dst];  self-loop n: dinv[n]^2
  agg     = sum over incoming edges of norm_e * x[src]         [N, 128]
  h       = leaky_relu(agg @ W1 + b1)                          [N, 256]
  pooled  = segment_mean(h, batch)                             [64, 256]
  z       = leaky_relu(concat(pooled, emb[cls]) @ W2 + b2)
  out     = z @ W3 + b3                                        [64, 1]

Sharding: graph-aligned.  batch is sorted, so graphs occupy contiguous node
ranges; each core owns 8 whole graphs (and the nodes/edge-destinations that
fall in them).  Pooling is therefore core-local — no collective at all; each
core runs the tiny MLP on its own 8 graphs and the host concatenates the
8x[8,1] results.

The irregular gather x[src] runs as HBM dma_gather (SWDGE descriptors)
spread over all 4 SWDGE queues so descriptor emission on the Q7 core-pairs
overlaps the SDMA drains.  Indices are signed-int16 offsets from row XOFF so
one gather covers all 50k source rows.  Self-loop contributions read
contiguous x rows (dense DMA) and are folded in on the Vector engine during
the PSUM evacuation.  The edge segment-sum runs on the PE as one-hot
(weight-scaled) matmuls; pooling is a count-scaled one-hot matmul
accumulated in PSUM.
"""

import numpy as np
import ml_dtypes
from collections import defaultdict

# ----------------------------------------------------------------- config
CFG = dict(
    N=50000, F=128, HID=256, G=64, NCLS=10,
    NCORES=8,
    XOFF=25000,           # gather base row (signed int16 offsets)
    ST_D=128,             # dsts per supertile (= matmul M)
    K=128,                # edge slots per chunk (matmul contraction)
    GRAN=2,               # supertiles per gather granule
    TAPER=4,              # trailing supertiles run as single-st granules
    NEG=0.2,
    NQ=4,                 # SWDGE queues used round-robin for gathers
    SPLIT=2,              # gathers per granule (parallel Q7 emission)
    NO_GATHER=0,          # debug: memset instead of dma_gather
    GDT="bf16",           # gather dtype: "f32" | "bf16"  (x rows + S weights)
    MMDT="bf16",          # downstream matmul dtype
)


def _np_dt(s):
    return {"f32": np.float32, "bf16": ml_dtypes.bfloat16}[s]


# ================================================================= host prep
class Prep:
    pass


def host_prep(inputs, cfg):
    """Integer/layout preprocessing + normalization weights.

    Returns per-core in_maps plus the static (core-independent) program
    structure.
    """
    N, F, G = cfg["N"], cfg["F"], cfg["G"]
    NC, XOFF = cfg["NCORES"], cfg["XOFF"]
    ST_D, K, GRAN = cfg["ST_D"], cfg["K"], cfg["GRAN"]
    GL = G // NC                                   # graphs per core

    x = np.asarray(inputs["x"], np.float32)
    ei = np.asarray(inputs["edge_index"], np.int64)
    ew = np.asarray(inputs["edge_weight"], np.float32)
    batch = np.asarray(inputs["batch"], np.int64)
    cls = np.asarray(inputs["class_labels"], np.int64)
    W1 = np.asarray(inputs["W1"], np.float32)
    b1 = np.asarray(inputs["b1"], np.float32)
    emb = np.asarray(inputs["emb"], np.float32)
    W2 = np.asarray(inputs["W2"], np.float32)
    b2 = np.asarray(inputs["b2"], np.float32)
    W3 = np.asarray(inputs["W3"], np.float32)
    b3 = np.asarray(inputs["b3"], np.float32)

    HID = W1.shape[1]
    EH = emb.shape[1]

    # --- normalization weights (scalar preprocessing, O(E)) -------------
    row = ei[0]
    col = ei[1]
    deg = np.zeros(N, np.float64)
    np.add.at(deg, col, ew.astype(np.float64))
    deg += 1.0
    dinv = 1.0 / np.sqrt(deg)
    a_w = (dinv[row] * ew.astype(np.float64) * dinv[col]).astype(np.float32)
    a_src = row
    a_dst = col
    selfw = (dinv * dinv).astype(np.float32)       # self-loop weights [N]

    # --- balanced graph->core assignment (LPT bin packing, 8 per core) ----
    gcnt = np.bincount(batch, minlength=G)           # nodes per graph
    gcum = np.concatenate([[0], np.cumsum(gcnt)])    # graph start offsets
    load = np.zeros(NC, np.int64)
    nfill = np.zeros(NC, np.int64)
    asg = [[] for _ in range(NC)]                    # graphs per core
    for g in np.argsort(-gcnt, kind="stable"):
        c = min((c for c in range(NC) if nfill[c] < GL),
                key=lambda c: load[c])
        asg[c].append(int(g))
        load[c] += gcnt[g]
        nfill[c] += 1
    nlists = []                                      # per-core node ids
    core_of_node = np.empty(N, np.int64)
    loc_of_node = np.empty(N, np.int64)
    for c in range(NC):
        nl = np.concatenate(
            [np.arange(gcum[g], gcum[g + 1]) for g in asg[c]])
        nlists.append(nl)
        core_of_node[nl] = c
        loc_of_node[nl] = np.arange(len(nl))
    Dmax = int(load.max())
    NST = -(-Dmax // ST_D)      # supertiles per core

    # --- bucket edges into (core, st) ------------------------------------
    core_of = core_of_node[a_dst]
    dst_loc = loc_of_node[a_dst]
    st_of = dst_loc // ST_D
    j_of = dst_loc % ST_D
    key = core_of * NST + st_of

    nbuckets = NC * NST
    cnt = np.bincount(key, minlength=nbuckets).reshape(NC, NST)
    starts = np.zeros(nbuckets + 1, np.int64)
    np.cumsum(cnt.reshape(-1), out=starts[1:])

    # Per-core x-row permutation: order nodes by first-use bucket so each
    # bucket's "home" sources occupy consecutive rows — their 256B gather
    # reads become sequential (HBM row-buffer hits).  Buckets are then
    # sorted by permuted id, making home reads ascending runs.
    perms = []
    seg_src = []   # per-core permuted signed src offsets, bucket-sorted
    seg_j = []
    seg_w = []
    for c in range(NC):
        mask = core_of == c
        st_c = st_of[mask]
        src_c = a_src[mask]
        j_c = j_of[mask]
        w_c = a_w[mask]
        o0 = np.lexsort((src_c, st_c))
        seq = src_c[o0]
        uniq, first_idx = np.unique(seq, return_index=True)
        order_nodes = uniq[np.argsort(first_idx)]
        perm = np.full(N, -1, np.int64)
        perm[order_nodes] = np.arange(len(order_nodes))
        rest = np.nonzero(perm < 0)[0]
        perm[rest] = len(order_nodes) + np.arange(len(rest))
        p_src = perm[src_c]
        o1 = np.lexsort((p_src, st_c))
        perms.append(perm)
        seg_src.append((p_src[o1] - XOFF).astype(np.int32))
        seg_j.append(j_c[o1].astype(np.int32))
        seg_w.append(w_c[o1])

    # static chunk counts per st: max over cores, >= 1; the +1 guarantees
    # at least one padding slot per bucket so every gather's trailing slot
    # is the non-negative padding index (the ucode drops trailing
    # negative-idx runs)
    CH = np.maximum((-(-(cnt + 1) // K)).max(axis=0), 1)   # [NST]

    # --- static program structure ---------------------------------------
    TAPER = cfg.get("TAPER", 0)
    grans = []
    st = 0
    while st < NST:
        n = 1 if st >= NST - TAPER else min(GRAN, NST - st)
        grans.append((st, n))
        st += n

    SELF = -1                                      # self-loop chunk marker
    gmeta = []
    mm_total = 0
    col_total = 0
    for (st0, nst) in grans:
        sts = list(range(st0, st0 + nst))
        nch = int(sum(CH[st] for st in sts))
        cimap = {}
        mms = []
        ci = 0
        for st in sts:
            for k in range(int(CH[st])):
                cimap[(st, k)] = ci
                mms.append((st, k))
                ci += 1
            mms.append((st, SELF))                 # diag(selfw) @ xself[st]
        gmeta.append(dict(st0=st0, nst=nst, nch=nch,
                          cimap=cimap, mms=mms,
                          mm_off=mm_total, col_off=col_total))
        mm_total += len(mms)
        col_total += nch * (K // 16)
    NMM = mm_total
    NCOLS = col_total

    static = dict(cfg=cfg, NST=NST, CH=CH, grans=grans, gmeta=gmeta,
                  NMM=NMM, NCOLS=NCOLS, HID=HID, EH=EH, GL=GL, asg=asg)

    # --- per-core tensors ------------------------------------------------
    gdt = _np_dt(cfg["GDT"])
    mmdt = _np_dt(cfg["MMDT"])

    counts = np.maximum(gcnt, 1).astype(np.float64)

    w2blk = np.zeros((128, 6 * 128), np.float32)
    for kk in range(3):
        for jj in range(2):
            w2blk[:, (kk * 2 + jj) * 128:(kk * 2 + jj + 1) * 128] = \
                W2[kk * 128:(kk + 1) * 128, jj * 128:(jj + 1) * 128]
    w3m = np.zeros((128, 2), np.float32)
    w3m[:, 0] = W3[0:128, 0]
    w3m[:, 1] = W3[128:256, 0]

    xg = np.ascontiguousarray(x).astype(gdt)         # gather/dense source

    in_maps = []
    for c in range(NC):
        cstart = starts[c * NST]
        srcl_s = seg_src[c]
        j_s = seg_j[c]
        w_s = seg_w[c]
        perm = perms[c]
        # permuted per-core x copy: new row perm[i] holds x[i]
        xgc = np.empty_like(xg)
        xgc[perm] = xg
        gidx = np.zeros((NCOLS, 16), np.int16)
        smat = np.zeros((128, NMM * ST_D), gdt)
        for gi, gm in enumerate(gmeta):
            nch = gm["nch"]
            slots_src = np.zeros((nch * K,), np.int32)
            for (st_k, ci) in gm["cimap"].items():
                st, k = st_k
                b = c * NST + st
                s0, s1 = starts[b] - cstart, starts[b + 1] - cstart
                seg = srcl_s[s0:s1]
                part = seg[k * K:(k + 1) * K]
                slots_src[ci * K: ci * K + len(part)] = part
            # gather ucode drops a trailing negative-idx run; gathers are
            # bucket-aligned and padding slots are 0 (>= 0), so this only
            # triggers if a bucket's final chunk is exactly full of
            # all-sub-XOFF sources — vanishingly unlikely with
            # ascending-src bucket order.
            ce = 0
            for st in range(gm["st0"], gm["st0"] + gm["nst"]):
                ce += int(CH[st])
                assert slots_src[ce * K - 1] >= 0, "trailing negative idx"
            c0 = gm["col_off"]
            gidx[c0:c0 + nch * (K // 16), :] = slots_src.reshape(-1, 16)
            # S matrices
            for mi, (st, k) in enumerate(gm["mms"]):
                mm = gm["mm_off"] + mi
                if k < 0:
                    # self-loop diag chunk: agg += diag(selfw) @ xself[st]
                    nl = nlists[c]
                    d0 = st * ST_D
                    nvalid = min(ST_D, max(0, len(nl) - d0))
                    rows = np.arange(nvalid)
                    smat[rows, mm * ST_D + rows] = \
                        selfw[nl[d0:d0 + nvalid]].astype(gdt)
                    continue
                b = c * NST + st
                s0, s1 = starts[b] - cstart, starts[b + 1] - cstart
                jj = j_s[s0:s1][k * K:(k + 1) * K]
                ww = w_s[s0:s1][k * K:(k + 1) * K]
                rows = np.arange(len(jj))
                smat[rows, mm * ST_D + jj] = ww.astype(gdt)
        gidx2 = np.tile(gidx.T, (8, 1))              # [128, NCOLS]

        nl = nlists[c]
        gslot = {g: j for j, g in enumerate(asg[c])}

        # pooling matrix [128, NST*GL], scaled by 1/count (mean pooling)
        pmat = np.zeros((128, NST * GL), np.float64)
        for stn in range(NST):
            d0 = stn * ST_D
            nv = min(ST_D, max(0, len(nl) - d0))
            for p in range(nv):
                g = batch[nl[d0 + p]]
                pmat[p, stn * GL + gslot[g]] = 1.0 / counts[g]

        # self-loop x rows: the core's local nodes in local order, padded
        xself = np.zeros((NST * ST_D, F), gdt)
        xself[:len(nl)] = xg[nl]

        # class embeddings for this core's graphs: clt [NCLS, GL]
        clt = np.zeros((cfg["NCLS"], GL), mmdt)
        clt[cls[asg[c]], np.arange(GL)] = 1.0

        m = dict(
            xg=xgc, xself=xself,
            gidx=np.ascontiguousarray(gidx2),
            smat=smat,
            pmat=pmat.astype(mmdt),
            w1=W1.astype(mmdt),
            w2blk=w2blk.astype(mmdt),
            w3=w3m.astype(mmdt),
            b1=b1.reshape(1, HID).astype(mmdt),
            b2=b2.reshape(1, HID).astype(mmdt),
            b3=b3.reshape(1, 1).astype(mmdt),
            embh=emb.astype(mmdt),
            clt=clt,
        )
        in_maps.append(m)

    prep = Prep()
    prep.static = static
    prep.in_maps = in_maps
    return prep


# ================================================================= builder
def build(static):
    import concourse.bass as bass
    from concourse import bacc, tile
    import concourse.mybir as mybir

    cfg = static["cfg"]
    N, F = cfg["N"], cfg["F"]
    XOFF = cfg["XOFF"]
    ST_D, K = cfg["ST_D"], cfg["K"]
    NST, CH, gmeta = static["NST"], static["CH"], static["gmeta"]
    NMM, NCOLS = static["NMM"], static["NCOLS"]
    HID, EH, GL = static["HID"], static["EH"], static["GL"]
    NCLS = cfg["NCLS"]
    NEG = cfg["NEG"]
    NQ = cfg["NQ"]

    gdt = {"f32": mybir.dt.float32, "bf16": mybir.dt.bfloat16}[cfg["GDT"]]
    mmdt = {"f32": mybir.dt.float32, "bf16": mybir.dt.bfloat16}[cfg["MMDT"]]
    f32 = mybir.dt.float32
    AF = mybir.ActivationFunctionType

    nc = bacc.Bacc(None, target_bir_lowering=False, debug=True,
                   num_swdge_queues=NQ)

    xg_d = nc.declare_dram_parameter("xg", [N, F], gdt, isOutput=False)
    xself_d = nc.declare_dram_parameter("xself", [NST * ST_D, F], gdt, isOutput=False)
    gidx_d = nc.declare_dram_parameter("gidx", [128, NCOLS], mybir.dt.int16, isOutput=False)
    smat_d = nc.declare_dram_parameter("smat", [128, NMM * ST_D], gdt, isOutput=False)
    pmat_d = nc.declare_dram_parameter("pmat", [128, NST * GL], mmdt, isOutput=False)
    w1_d = nc.declare_dram_parameter("w1", [F, HID], mmdt, isOutput=False)
    w2_d = nc.declare_dram_parameter("w2blk", [128, 6 * 128], mmdt, isOutput=False)
    w3_d = nc.declare_dram_parameter("w3", [128, 2], mmdt, isOutput=False)
    b1_d = nc.declare_dram_parameter("b1", [1, HID], mmdt, isOutput=False)
    b2_d = nc.declare_dram_parameter("b2", [1, HID], mmdt, isOutput=False)
    b3_d = nc.declare_dram_parameter("b3", [1, 1], mmdt, isOutput=False)
    emb_d = nc.declare_dram_parameter("embh", [NCLS, EH], mmdt, isOutput=False)
    clt_d = nc.declare_dram_parameter("clt", [NCLS, GL], mmdt, isOutput=False)
    out_d = nc.declare_dram_parameter("out", [1, GL], f32, isOutput=True)

    iden_np = np.eye(128, dtype=_np_dt(cfg["MMDT"]))
    iden_d = nc.inline_tensor(iden_np, name="iden")

    with tile.TileContext(nc) as tc:
        with (
            tc.tile_pool(name="const", bufs=1) as constp,
            tc.tile_pool(name="gix", bufs=6) as gixp,
            tc.tile_pool(name="gat", bufs=6) as gatp,
            tc.tile_pool(name="xsf", bufs=6) as xsfp,
            tc.tile_pool(name="smp", bufs=3) as smp,
            tc.tile_pool(name="work", bufs=3) as workp,
            tc.tile_pool(name="ps_agg", bufs=2, space="PSUM") as ps_agg,
            tc.tile_pool(name="ps_t", bufs=2, space="PSUM") as ps_t,
            tc.tile_pool(name="ps_h", bufs=2, space="PSUM") as ps_h,
            tc.tile_pool(name="ps_pool", bufs=1, space="PSUM") as ps_pool,
        ):
            # ---- persistent SBUF loads
            pmat_sb = constp.tile([128, NST * GL], mmdt)
            nc.sync.dma_start(out=pmat_sb[:, :], in_=pmat_d[:, :])
            w1_sb = constp.tile([F, HID], mmdt)
            nc.sync.dma_start(out=w1_sb[:, :], in_=w1_d[:, :])
            iden_sb = constp.tile([128, 128], mmdt)
            nc.sync.dma_start(out=iden_sb[:, :], in_=iden_d[:, :])
            b1_sb = constp.tile([1, HID], mmdt)
            nc.sync.dma_start(out=b1_sb[:, :], in_=b1_d[:, :])
            ones_sb = constp.tile([1, 128], mmdt)
            nc.vector.memset(ones_sb[:, :], 1.0)

            pooled_ps = ps_pool.tile([GL, HID], f32)

            # ---------------- main loop over granules
            qn = 0
            for gi, gm in enumerate(gmeta):
                st0, nst = gm["st0"], gm["nst"]
                nch = gm["nch"]
                c0 = gm["col_off"]
                gt = gatp.tile([128, nch, F], gdt, tag="gt")
                gx = gixp.tile([128, nch * (K // 16)], mybir.dt.int16, tag="gx")
                nc.scalar.dma_start(
                    out=gx[:, :],
                    in_=gidx_d[:, c0:c0 + nch * (K // 16)])
                if cfg.get("NO_GATHER"):
                    nc.vector.memset(gt[:, :, :], 0.125)
                else:
                    # one gather per supertile (bucket-aligned so the
                    # trailing-negative-idx drop can't hit real slots),
                    # spread over SWDGE queues for parallel Q7 emission
                    ca = 0
                    for st in range(st0, st0 + nst):
                        cb = ca + int(CH[st])
                        nc.gpsimd.dma_gather(
                            gt[:, ca:cb, :], xg_d[XOFF:, :],
                            gx[:, ca * (K // 16):cb * (K // 16)],
                            num_idxs=(cb - ca) * K,
                            num_idxs_reg=(cb - ca) * K,
                            elem_size=F, single_packet=bool(cfg.get("SP1", 0)),
                            queue_num=qn % NQ)
                        qn += 1
                        ca = cb

                # dense self-loop rows for these supertiles
                xsf = xsfp.tile([128, nst, F], gdt, tag="xsf")
                for sti in range(nst):
                    nc.scalar.dma_start(
                        out=xsf[:, sti, :],
                        in_=xself_d[(st0 + sti) * ST_D:(st0 + sti + 1) * ST_D, :])

                nmm_g = len(gm["mms"])
                sm_sb = smp.tile([128, nmm_g * ST_D], gdt, tag="sm")
                m0 = gm["mm_off"]
                nc.sync.dma_start(
                    out=sm_sb[:, :],
                    in_=smat_d[:, m0 * ST_D:(m0 + nmm_g) * ST_D])

                by_st = defaultdict(list)
                for mi, (st, k) in enumerate(gm["mms"]):
                    by_st[st].append((mi, k))

                for sti, st in enumerate(range(st0, st0 + nst)):
                    # S-matmuls with gathered rows STATIONARY: the PSUM
                    # tile accumulates agg^T = gt.T @ S directly in the
                    # [feature, dst] layout the W1 matmul wants as lhsT —
                    # no transpose step needed.
                    agg = ps_agg.tile([128, F], f32, tag="agg")
                    lst = by_st[st]
                    for i, (mi, k) in enumerate(lst):
                        if k < 0:
                            lhsT = xsf[:, sti, :]    # self-loop diag chunk
                        else:
                            lhsT = gt[:, gm["cimap"][(st, k)], :]
                        nc.tensor.matmul(
                            agg[:, :],
                            lhsT=lhsT,
                            rhs=sm_sb[:, mi * ST_D:(mi + 1) * ST_D],
                            start=(i == 0), stop=(i == len(lst) - 1))
                    aggT_sb = workp.tile([128, 128], mmdt, tag="aggT_sb")
                    nc.vector.tensor_copy(out=aggT_sb[:, :], in_=agg[:, :])
                    # W1 + b1
                    h_ps = ps_h.tile([128, HID], f32, tag="h")
                    nc.tensor.matmul(h_ps[:, :], lhsT=aggT_sb[:, :],
                                     rhs=w1_sb[:, :], start=True, stop=False)
                    nc.tensor.matmul(h_ps[:, :], lhsT=ones_sb[:, 0:128],
                                     rhs=b1_sb[:, :], start=False, stop=True)
                    # leaky relu -> sbuf
                    hr_sb = workp.tile([128, HID], f32, tag="hr_sb")
                    nc.scalar.activation(hr_sb[:, :], h_ps[:, :], AF.Relu,
                                         scale=1.0 - NEG)
                    h_sb = workp.tile([128, HID], mmdt, tag="h_sb")
                    nc.vector.scalar_tensor_tensor(
                        h_sb[:, :], in0=h_ps[:, :], scalar=NEG,
                        in1=hr_sb[:, :], op0=mybir.AluOpType.mult,
                        op1=mybir.AluOpType.add)
                    # mean-pool accumulate (pmat carries 1/count)
                    nc.tensor.matmul(
                        pooled_ps[:, :],
                        lhsT=pmat_sb[:, st * GL:(st + 1) * GL],
                        rhs=h_sb[:, :],
                        start=(st == 0), stop=(st == NST - 1),
                        skip_group_check=True)

            # ---------------- tail: core-local MLP on GL graphs
            pm_sb = workp.tile([GL, HID], mmdt, tag="pm")
            nc.vector.tensor_copy(out=pm_sb[:, :], in_=pooled_ps[:, :])

            # transpose pooled -> z^T rows [128, GL] halves
            zt = []
            for jj in range(HID // 128):
                tp = ps_t.tile([128, GL], mmdt, tag="aggT")
                nc.tensor.transpose(tp[:, :], pm_sb[:, jj * 128:(jj + 1) * 128],
                                    iden_sb[0:GL, 0:GL])
                t_sb = workp.tile([128, GL], mmdt, tag=f"zt{jj}")
                nc.scalar.copy(out=t_sb[:, :], in_=tp[:, :])
                zt.append(t_sb)
            # class-embedding^T [EH, GL]
            emb_sb = workp.tile([NCLS, EH], mmdt, tag="emb")
            nc.sync.dma_start(out=emb_sb[:, :], in_=emb_d[:, :])
            clt_sb = workp.tile([NCLS, GL], mmdt, tag="clt")
            nc.sync.dma_start(out=clt_sb[:, :], in_=clt_d[:, :])
            ce_ps = ps_t.tile([EH, GL], f32, tag="aggT")
            nc.tensor.matmul(ce_ps[:, :], lhsT=emb_sb[:, :], rhs=clt_sb[:, :],
                             start=True, stop=True)
            ce_sb = workp.tile([EH, GL], mmdt, tag="ce_sb")
            nc.scalar.copy(out=ce_sb[:, :], in_=ce_ps[:, :])
            zt.append(ce_sb)

            # W2: z2^T[128j] = sum_k W2blk[k,j].T @ zt[k]
            w2_sb = workp.tile([128, 6 * 128], mmdt, tag="w2")
            nc.sync.dma_start(out=w2_sb[:, :], in_=w2_d[:, :])
            b2_sb = workp.tile([1, HID], mmdt, tag="b2")
            nc.sync.dma_start(out=b2_sb[:, :], in_=b2_d[:, :])
            ones_g = workp.tile([1, GL], mmdt, tag="onesg")
            nc.vector.memset(ones_g[:, :], 1.0)
            nk = (HID + EH) // 128
            z2 = []
            for jj in range(2):
                zp = ps_h.tile([128, GL], f32, tag="h")
                for kk in range(nk):
                    nc.tensor.matmul(
                        zp[:, :],
                        lhsT=w2_sb[:, (kk * 2 + jj) * 128:(kk * 2 + jj + 1) * 128],
                        rhs=zt[kk][:, :], start=(kk == 0), stop=False)
                nc.tensor.matmul(zp[:, :], lhsT=b2_sb[:, jj * 128:(jj + 1) * 128],
                                 rhs=ones_g[:, :], start=False, stop=True)
                zr_sb = workp.tile([128, GL], f32, tag="zr_sb")
                nc.scalar.activation(zr_sb[:, :], zp[:, :], AF.Relu,
                                     scale=1.0 - NEG)
                z_sb = workp.tile([128, GL], mmdt, tag=f"z2sb{jj}")
                nc.vector.scalar_tensor_tensor(
                    z_sb[:, :], in0=zp[:, :], scalar=NEG, in1=zr_sb[:, :],
                    op0=mybir.AluOpType.mult, op1=mybir.AluOpType.add)
                z2.append(z_sb)

            w3_sb = workp.tile([128, 2], mmdt, tag="w3")
            nc.sync.dma_start(out=w3_sb[:, :], in_=w3_d[:, :])
            b3_sb = workp.tile([1, 1], mmdt, tag="b3")
            nc.sync.dma_start(out=b3_sb[:, :], in_=b3_d[:, :])
            op = ps_h.tile([1, GL], f32, tag="h")
            for jj in range(2):
                nc.tensor.matmul(op[:, :], lhsT=w3_sb[:, jj:jj + 1],
                                 rhs=z2[jj][:, :], start=(jj == 0), stop=False)
            nc.tensor.matmul(op[:, :], lhsT=b3_sb[:, :], rhs=ones_g[:, :],
                             start=False, stop=True)
            o_sb = workp.tile([1, GL], f32, tag="osb")
            nc.vector.tensor_copy(out=o_sb[:, :], in_=op[:, :])
            nc.sync.dma_start(out=out_d[:, :], in_=o_sb[:, :])

    return nc


# ================================================================= runner
def _run(inputs, cfg=None, trace=False):
    from concourse.bass_utils import run_bass_kernel_spmd
    cfg = dict(CFG if cfg is None else cfg)
    prep = host_prep(inputs, cfg)
    nc = build(prep.static)
    nc.finalize()
    res = run_bass_kernel_spmd(
        nc, prep.in_maps, core_ids=list(range(cfg["NCORES"])), trace=trace)
    G = cfg["G"]
    out = np.zeros((G, 1), np.float32)
    for c, r in enumerate(res.results):
        oc = np.asarray(r["out"], np.float32).reshape(-1)
        for j, g in enumerate(prep.static["asg"][c]):
            out[g, 0] = oc[j]
    return out, res


def kernel(**inputs):
    out, _ = _run(inputs)
    return out


# revision 43
# speedup vs baseline: 1.1421x; 1.0415x over previous
"""GCN discriminator kernel for Trainium2 (8 NeuronCores, SPMD).

Math (matching the reference):
  deg[n]  = sum_{e: dst=n} w_e + 1
  dinv    = 1/sqrt(deg)
  norm_e  = dinv[src]*w_e*dinv[dst];  self-loop n: dinv[n]^2
  agg     = sum over incoming edges of norm_e * x[src]         [N, 128]
  h       = leaky_relu(agg @ W1 + b1)                          [N, 256]
  pooled  = segment_mean(h, batch)                             [64, 256]
  z       = leaky_relu(concat(pooled, emb[cls]) @ W2 + b2)
  out     = z @ W3 + b3                                        [64, 1]

Sharding: graph-aligned.  batch is sorted, so graphs occupy contiguous node
ranges; each core owns 8 whole graphs (and the nodes/edge-destinations that
fall in them).  Pooling is therefore core-local — no collective at all; each
core runs the tiny MLP on its own 8 graphs and the host concatenates the
8x[8,1] results.

The irregular gather x[src] runs as HBM dma_gather (SWDGE descriptors)
spread over all 4 SWDGE queues so descriptor emission on the Q7 core-pairs
overlaps the SDMA drains.  Indices are signed-int16 offsets from row XOFF so
one gather covers all 50k source rows.  Self-loop contributions read
contiguous x rows (dense DMA) and are folded in on the Vector engine during
the PSUM evacuation.  The edge segment-sum runs on the PE as one-hot
(weight-scaled) matmuls; pooling is a count-scaled one-hot matmul
accumulated in PSUM.
"""

import numpy as np
import ml_dtypes
from collections import defaultdict

# ----------------------------------------------------------------- config
CFG = dict(
    N=50000, F=128, HID=256, G=64, NCLS=10,
    NCORES=8,
    XOFF=25000,           # gather base row (signed int16 offsets)
    ST_D=128,             # dsts per supertile (= matmul M)
    K=128,                # edge slots per chunk (matmul contraction)
    GRAN=2,               # supertiles per gather granule
    TAPER=4,              # trailing supertiles run as single-st granules
    NEG=0.2,
    NQ=4,                 # SWDGE queues used round-robin for gathers
    SPLIT=2,              # gathers per granule (parallel Q7 emission)
    NO_GATHER=0,          # debug: memset instead of dma_gather
    GDT="bf16",           # gather dtype: "f32" | "bf16"  (x rows + S weights)
    MMDT="bf16",          # downstream matmul dtype
)


def _np_dt(s):
    return {"f32": np.float32, "bf16": ml_dtypes.bfloat16}[s]


# ================================================================= host prep
class Prep:
    pass


def host_prep(inputs, cfg):
    """Integer/layout preprocessing + normalization weights.

    Returns per-core in_maps plus the static (core-independent) program
    structure.
    """
    N, F, G = cfg["N"], cfg["F"], cfg["G"]
    NC, XOFF = cfg["NCORES"], cfg["XOFF"]
    ST_D, K, GRAN = cfg["ST_D"], cfg["K"], cfg["GRAN"]
    GL = G // NC                                   # graphs per core

    x = np.asarray(inputs["x"], np.float32)
    ei = np.asarray(inputs["edge_index"], np.int64)
    ew = np.asarray(inputs["edge_weight"], np.float32)
    batch = np.asarray(inputs["batch"], np.int64)
    cls = np.asarray(inputs["class_labels"], np.int64)
    W1 = np.asarray(inputs["W1"], np.float32)
    b1 = np.asarray(inputs["b1"], np.float32)
    emb = np.asarray(inputs["emb"], np.float32)
    W2 = np.asarray(inputs["W2"], np.float32)
    b2 = np.asarray(inputs["b2"], np.float32)
    W3 = np.asarray(inputs["W3"], np.float32)
    b3 = np.asarray(inputs["b3"], np.float32)

    HID = W1.shape[1]
    EH = emb.shape[1]

    # --- normalization weights (scalar preprocessing, O(E)) -------------
    row = ei[0]
    col = ei[1]
    deg = np.zeros(N, np.float64)
    np.add.at(deg, col, ew.astype(np.float64))
    deg += 1.0
    dinv = 1.0 / np.sqrt(deg)
    a_w = (dinv[row] * ew.astype(np.float64) * dinv[col]).astype(np.float32)
    a_src = row
    a_dst = col
    selfw = (dinv * dinv).astype(np.float32)       # self-loop weights [N]

    # --- balanced graph->core assignment (LPT bin packing, 8 per core) ----
    gcnt = np.bincount(batch, minlength=G)           # nodes per graph
    gcum = np.concatenate([[0], np.cumsum(gcnt)])    # graph start offsets
    load = np.zeros(NC, np.int64)
    nfill = np.zeros(NC, np.int64)
    asg = [[] for _ in range(NC)]                    # graphs per core
    for g in np.argsort(-gcnt, kind="stable"):
        c = min((c for c in range(NC) if nfill[c] < GL),
                key=lambda c: load[c])
        asg[c].append(int(g))
        load[c] += gcnt[g]
        nfill[c] += 1
    nlists = []                                      # per-core node ids
    core_of_node = np.empty(N, np.int64)
    loc_of_node = np.empty(N, np.int64)
    for c in range(NC):
        nl = np.concatenate(
            [np.arange(gcum[g], gcum[g + 1]) for g in asg[c]])
        nlists.append(nl)
        core_of_node[nl] = c
        loc_of_node[nl] = np.arange(len(nl))
    Dmax = int(load.max())
    NST = -(-Dmax // ST_D)      # supertiles per core

    # --- bucket edges into (core, st) ------------------------------------
    core_of = core_of_node[a_dst]
    dst_loc = loc_of_node[a_dst]
    st_of = dst_loc // ST_D
    j_of = dst_loc % ST_D
    key = core_of * NST + st_of

    nbuckets = NC * NST
    cnt = np.bincount(key, minlength=nbuckets).reshape(NC, NST)
    starts = np.zeros(nbuckets + 1, np.int64)
    np.cumsum(cnt.reshape(-1), out=starts[1:])

    # Per-core x-row permutation: order nodes by first-use bucket so each
    # bucket's "home" sources occupy consecutive rows — their 256B gather
    # reads become sequential (HBM row-buffer hits).  Buckets are then
    # sorted by permuted id, making home reads ascending runs.
    perms = []
    seg_src = []   # per-core permuted signed src offsets, bucket-sorted
    seg_j = []
    seg_w = []
    for c in range(NC):
        mask = core_of == c
        st_c = st_of[mask]
        src_c = a_src[mask]
        j_c = j_of[mask]
        w_c = a_w[mask]
        o0 = np.lexsort((src_c, st_c))
        seq = src_c[o0]
        uniq, first_idx = np.unique(seq, return_index=True)
        order_nodes = uniq[np.argsort(first_idx)]
        perm = np.full(N, -1, np.int64)
        perm[order_nodes] = np.arange(len(order_nodes))
        rest = np.nonzero(perm < 0)[0]
        perm[rest] = len(order_nodes) + np.arange(len(rest))
        p_src = perm[src_c]
        o1 = np.lexsort((p_src, st_c))
        perms.append(perm)
        seg_src.append((p_src[o1] - XOFF).astype(np.int32))
        seg_j.append(j_c[o1].astype(np.int32))
        seg_w.append(w_c[o1])

    # static chunk counts per st: max over cores, >= 1; the +1 guarantees
    # at least one padding slot per bucket so every gather's trailing slot
    # is the non-negative padding index (the ucode drops trailing
    # negative-idx runs)
    CH = np.maximum((-(-(cnt + 1) // K)).max(axis=0), 1)   # [NST]

    # --- static program structure ---------------------------------------
    TAPER = cfg.get("TAPER", 0)
    grans = []
    st = 0
    while st < NST:
        n = 1 if st >= NST - TAPER else min(GRAN, NST - st)
        grans.append((st, n))
        st += n

    SELF = -1                                      # self-loop chunk marker
    gmeta = []
    mm_total = 0
    col_total = 0
    for (st0, nst) in grans:
        sts = list(range(st0, st0 + nst))
        nch = int(sum(CH[st] for st in sts))
        cimap = {}
        mms = []
        ci = 0
        for st in sts:
            for k in range(int(CH[st])):
                cimap[(st, k)] = ci
                mms.append((st, k))
                ci += 1
            mms.append((st, SELF))                 # diag(selfw) @ xself[st]
        gmeta.append(dict(st0=st0, nst=nst, nch=nch,
                          cimap=cimap, mms=mms,
                          mm_off=mm_total, col_off=col_total))
        mm_total += len(mms)
        col_total += nch * (K // 16)
    NMM = mm_total
    NCOLS = col_total

    static = dict(cfg=cfg, NST=NST, CH=CH, grans=grans, gmeta=gmeta,
                  NMM=NMM, NCOLS=NCOLS, HID=HID, EH=EH, GL=GL, asg=asg)

    # --- per-core tensors ------------------------------------------------
    gdt = _np_dt(cfg["GDT"])
    mmdt = _np_dt(cfg["MMDT"])

    counts = np.maximum(gcnt, 1).astype(np.float64)

    w2blk = np.zeros((128, 6 * 128), np.float32)
    for kk in range(3):
        for jj in range(2):
            w2blk[:, (kk * 2 + jj) * 128:(kk * 2 + jj + 1) * 128] = \
                W2[kk * 128:(kk + 1) * 128, jj * 128:(jj + 1) * 128]
    w3m = np.zeros((128, 2), np.float32)
    w3m[:, 0] = W3[0:128, 0]
    w3m[:, 1] = W3[128:256, 0]

    xg = np.ascontiguousarray(x).astype(gdt)         # gather/dense source

    in_maps = []
    for c in range(NC):
        cstart = starts[c * NST]
        srcl_s = seg_src[c]
        j_s = seg_j[c]
        w_s = seg_w[c]
        perm = perms[c]
        # permuted per-core x copy: new row perm[i] holds x[i]
        xgc = np.empty_like(xg)
        xgc[perm] = xg
        gidx = np.zeros((NCOLS, 16), np.int16)
        smat = np.zeros((128, NMM * ST_D), gdt)
        for gi, gm in enumerate(gmeta):
            nch = gm["nch"]
            slots_src = np.zeros((nch * K,), np.int32)
            for (st_k, ci) in gm["cimap"].items():
                st, k = st_k
                b = c * NST + st
                s0, s1 = starts[b] - cstart, starts[b + 1] - cstart
                seg = srcl_s[s0:s1]
                part = seg[k * K:(k + 1) * K]
                slots_src[ci * K: ci * K + len(part)] = part
            # gather ucode drops a trailing negative-idx run; gathers are
            # bucket-aligned and padding slots are 0 (>= 0), so this only
            # triggers if a bucket's final chunk is exactly full of
            # all-sub-XOFF sources — vanishingly unlikely with
            # ascending-src bucket order.
            ce = 0
            for st in range(gm["st0"], gm["st0"] + gm["nst"]):
                ce += int(CH[st])
                assert slots_src[ce * K - 1] >= 0, "trailing negative idx"
            c0 = gm["col_off"]
            gidx[c0:c0 + nch * (K // 16), :] = slots_src.reshape(-1, 16)
            # S matrices
            for mi, (st, k) in enumerate(gm["mms"]):
                mm = gm["mm_off"] + mi
                if k < 0:
                    # self-loop diag chunk: agg += diag(selfw) @ xself[st]
                    nl = nlists[c]
                    d0 = st * ST_D
                    nvalid = min(ST_D, max(0, len(nl) - d0))
                    rows = np.arange(nvalid)
                    smat[rows, mm * ST_D + rows] = \
                        selfw[nl[d0:d0 + nvalid]].astype(gdt)
                    continue
                b = c * NST + st
                s0, s1 = starts[b] - cstart, starts[b + 1] - cstart
                jj = j_s[s0:s1][k * K:(k + 1) * K]
                ww = w_s[s0:s1][k * K:(k + 1) * K]
                rows = np.arange(len(jj))
                smat[rows, mm * ST_D + jj] = ww.astype(gdt)
        gidx2 = np.tile(gidx.T, (8, 1))              # [128, NCOLS]

        nl = nlists[c]
        gslot = {g: j for j, g in enumerate(asg[c])}

        # pooling matrix [128, NST*GL], scaled by 1/count (mean pooling)
        pmat = np.zeros((128, NST * GL), np.float64)
        for stn in range(NST):
            d0 = stn * ST_D
            nv = min(ST_D, max(0, len(nl) - d0))
            for p in range(nv):
                g = batch[nl[d0 + p]]
                pmat[p, stn * GL + gslot[g]] = 1.0 / counts[g]

        # self-loop x rows: the core's local nodes in local order, padded
        xself = np.zeros((NST * ST_D, F), gdt)
        xself[:len(nl)] = xg[nl]

        # class embeddings for this core's graphs: clt [NCLS, GL]
        clt = np.zeros((cfg["NCLS"], GL), mmdt)
        clt[cls[asg[c]], np.arange(GL)] = 1.0

        m = dict(
            xg=xgc, xself=xself,
            gidx=np.ascontiguousarray(gidx2),
            smat=smat,
            pmat=pmat.astype(mmdt),
            w1=W1.astype(mmdt),
            w2blk=w2blk.astype(mmdt),
            w3=w3m.astype(mmdt),
            b1=b1.reshape(1, HID).astype(mmdt),
            b2=b2.reshape(1, HID).astype(mmdt),
            b3=b3.reshape(1, 1).astype(mmdt),
            embh=emb.astype(mmdt),
            clt=clt,
        )
        in_maps.append(m)

    prep = Prep()
    prep.static = static
    prep.in_maps = in_maps
    return prep


# ================================================================= builder
def build(static):
    import concourse.bass as bass
    from concourse import bacc, tile
    import concourse.mybir as mybir

    cfg = static["cfg"]
    N, F = cfg["N"], cfg["F"]
    XOFF = cfg["XOFF"]
    ST_D, K = cfg["ST_D"], cfg["K"]
    NST, CH, gmeta = static["NST"], static["CH"], static["gmeta"]
    NMM, NCOLS = static["NMM"], static["NCOLS"]
    HID, EH, GL = static["HID"], static["EH"], static["GL"]
    NCLS = cfg["NCLS"]
    NEG = cfg["NEG"]
    NQ = cfg["NQ"]

    gdt = {"f32": mybir.dt.float32, "bf16": mybir.dt.bfloat16}[cfg["GDT"]]
    mmdt = {"f32": mybir.dt.float32, "bf16": mybir.dt.bfloat16}[cfg["MMDT"]]
    f32 = mybir.dt.float32
    AF = mybir.ActivationFunctionType

    nc = bacc.Bacc(None, target_bir_lowering=False, debug=True,
                   num_swdge_queues=NQ)

    xg_d = nc.declare_dram_parameter("xg", [N, F], gdt, isOutput=False)
    xself_d = nc.declare_dram_parameter("xself", [NST * ST_D, F], gdt, isOutput=False)
    gidx_d = nc.declare_dram_parameter("gidx", [128, NCOLS], mybir.dt.int16, isOutput=False)
    smat_d = nc.declare_dram_parameter("smat", [128, NMM * ST_D], gdt, isOutput=False)
    pmat_d = nc.declare_dram_parameter("pmat", [128, NST * GL], mmdt, isOutput=False)
    w1_d = nc.declare_dram_parameter("w1", [F, HID], mmdt, isOutput=False)
    w2_d = nc.declare_dram_parameter("w2blk", [128, 6 * 128], mmdt, isOutput=False)
    w3_d = nc.declare_dram_parameter("w3", [128, 2], mmdt, isOutput=False)
    b1_d = nc.declare_dram_parameter("b1", [1, HID], mmdt, isOutput=False)
    b2_d = nc.declare_dram_parameter("b2", [1, HID], mmdt, isOutput=False)
    b3_d = nc.declare_dram_parameter("b3", [1, 1], mmdt, isOutput=False)
    emb_d = nc.declare_dram_parameter("embh", [NCLS, EH], mmdt, isOutput=False)
    clt_d = nc.declare_dram_parameter("clt", [NCLS, GL], mmdt, isOutput=False)
    out_d = nc.declare_dram_parameter("out", [1, GL], f32, isOutput=True)

    iden_np = np.eye(128, dtype=_np_dt(cfg["MMDT"]))
    iden_d = nc.inline_tensor(iden_np, name="iden")

    with tile.TileContext(nc) as tc:
        with (
            tc.tile_pool(name="const", bufs=1) as constp,
            tc.tile_pool(name="gix", bufs=6) as gixp,
            tc.tile_pool(name="gat", bufs=6) as gatp,
            tc.tile_pool(name="xsf", bufs=6) as xsfp,
            tc.tile_pool(name="smp", bufs=3) as smp,
            tc.tile_pool(name="work", bufs=3) as workp,
            tc.tile_pool(name="ps_agg", bufs=2, space="PSUM") as ps_agg,
            tc.tile_pool(name="ps_t", bufs=2, space="PSUM") as ps_t,
            tc.tile_pool(name="ps_h", bufs=2, space="PSUM") as ps_h,
            tc.tile_pool(name="ps_pool", bufs=1, space="PSUM") as ps_pool,
        ):
            # ---- persistent SBUF loads
            pmat_sb = constp.tile([128, NST * GL], mmdt)
            nc.sync.dma_start(out=pmat_sb[:, :], in_=pmat_d[:, :])
            w1_sb = constp.tile([F, HID], mmdt)
            nc.sync.dma_start(out=w1_sb[:, :], in_=w1_d[:, :])
            iden_sb = constp.tile([128, 128], mmdt)
            nc.sync.dma_start(out=iden_sb[:, :], in_=iden_d[:, :])
            b1_sb = constp.tile([1, HID], mmdt)
            nc.sync.dma_start(out=b1_sb[:, :], in_=b1_d[:, :])
            ones_sb = constp.tile([1, 128], mmdt)
            nc.vector.memset(ones_sb[:, :], 1.0)

            pooled_ps = ps_pool.tile([GL, HID], f32)

            # ---------------- main loop over granules
            qn = 0
            for gi, gm in enumerate(gmeta):
                st0, nst = gm["st0"], gm["nst"]
                nch = gm["nch"]
                c0 = gm["col_off"]
                gx = gixp.tile([128, nch * (K // 16)], mybir.dt.int16, tag="gx")
                nc.scalar.dma_start(
                    out=gx[:, :],
                    in_=gidx_d[:, c0:c0 + nch * (K // 16)])
                # one gather AND one destination tile per supertile
                # (bucket-aligned so the trailing-negative-idx drop can't
                # hit real slots; separate tiles so the paired gathers have
                # no same-tile write dependency), spread over SWDGE queues
                # for parallel Q7 emission
                gtl = []
                ca = 0
                for sti, st in enumerate(range(st0, st0 + nst)):
                    nck = int(CH[st])
                    gtt = gatp.tile([128, nck, F], gdt, tag=f"gt{sti}",
                                    name="gtt")
                    if cfg.get("NO_GATHER"):
                        nc.vector.memset(gtt[:, :, :], 0.125)
                    else:
                        nc.gpsimd.dma_gather(
                            gtt[:, :, :], xg_d[XOFF:, :],
                            gx[:, ca * (K // 16):(ca + nck) * (K // 16)],
                            num_idxs=nck * K,
                            num_idxs_reg=nck * K,
                            elem_size=F, single_packet=bool(cfg.get("SP1", 0)),
                            queue_num=qn % NQ)
                        qn += 1
                    gtl.append(gtt)
                    ca += nck

                # dense self-loop rows for these supertiles
                xsf = xsfp.tile([128, nst, F], gdt, tag="xsf")
                for sti in range(nst):
                    nc.scalar.dma_start(
                        out=xsf[:, sti, :],
                        in_=xself_d[(st0 + sti) * ST_D:(st0 + sti + 1) * ST_D, :])

                nmm_g = len(gm["mms"])
                sm_sb = smp.tile([128, nmm_g * ST_D], gdt, tag="sm")
                m0 = gm["mm_off"]
                nc.sync.dma_start(
                    out=sm_sb[:, :],
                    in_=smat_d[:, m0 * ST_D:(m0 + nmm_g) * ST_D])

                by_st = defaultdict(list)
                for mi, (st, k) in enumerate(gm["mms"]):
                    by_st[st].append((mi, k))

                for sti, st in enumerate(range(st0, st0 + nst)):
                    # S-matmuls with gathered rows STATIONARY: the PSUM
                    # tile accumulates agg^T = gt.T @ S directly in the
                    # [feature, dst] layout the W1 matmul wants as lhsT —
                    # no transpose step needed.
                    agg = ps_agg.tile([128, F], f32, tag="agg")
                    lst = by_st[st]
                    for i, (mi, k) in enumerate(lst):
                        if k < 0:
                            lhsT = xsf[:, sti, :]    # self-loop diag chunk
                        else:
                            lhsT = gtl[sti][:, k, :]
                        nc.tensor.matmul(
                            agg[:, :],
                            lhsT=lhsT,
                            rhs=sm_sb[:, mi * ST_D:(mi + 1) * ST_D],
                            start=(i == 0), stop=(i == len(lst) - 1))
                    aggT_sb = workp.tile([128, 128], mmdt, tag="aggT_sb")
                    nc.vector.tensor_copy(out=aggT_sb[:, :], in_=agg[:, :])
                    # W1 + b1
                    h_ps = ps_h.tile([128, HID], f32, tag="h")
                    nc.tensor.matmul(h_ps[:, :], lhsT=aggT_sb[:, :],
                                     rhs=w1_sb[:, :], start=True, stop=False)
                    nc.tensor.matmul(h_ps[:, :], lhsT=ones_sb[:, 0:128],
                                     rhs=b1_sb[:, :], start=False, stop=True)
                    # leaky relu -> sbuf
                    hr_sb = workp.tile([128, HID], f32, tag="hr_sb")
                    nc.scalar.activation(hr_sb[:, :], h_ps[:, :], AF.Relu,
                                         scale=1.0 - NEG)
                    h_sb = workp.tile([128, HID], mmdt, tag="h_sb")
                    nc.vector.scalar_tensor_tensor(
                        h_sb[:, :], in0=h_ps[:, :], scalar=NEG,
                        in1=hr_sb[:, :], op0=mybir.AluOpType.mult,
                        op1=mybir.AluOpType.add)
                    # mean-pool accumulate (pmat carries 1/count)
                    nc.tensor.matmul(
                        pooled_ps[:, :],
                        lhsT=pmat_sb[:, st * GL:(st + 1) * GL],
                        rhs=h_sb[:, :],
                        start=(st == 0), stop=(st == NST - 1),
                        skip_group_check=True)

            # ---------------- tail: core-local MLP on GL graphs
            pm_sb = workp.tile([GL, HID], mmdt, tag="pm")
            nc.vector.tensor_copy(out=pm_sb[:, :], in_=pooled_ps[:, :])

            # transpose pooled -> z^T rows [128, GL] halves
            zt = []
            for jj in range(HID // 128):
                tp = ps_t.tile([128, GL], mmdt, tag="aggT")
                nc.tensor.transpose(tp[:, :], pm_sb[:, jj * 128:(jj + 1) * 128],
                                    iden_sb[0:GL, 0:GL])
                t_sb = workp.tile([128, GL], mmdt, tag=f"zt{jj}")
                nc.scalar.copy(out=t_sb[:, :], in_=tp[:, :])
                zt.append(t_sb)
            # class-embedding^T [EH, GL]
            emb_sb = workp.tile([NCLS, EH], mmdt, tag="emb")
            nc.sync.dma_start(out=emb_sb[:, :], in_=emb_d[:, :])
            clt_sb = workp.tile([NCLS, GL], mmdt, tag="clt")
            nc.sync.dma_start(out=clt_sb[:, :], in_=clt_d[:, :])
            ce_ps = ps_t.tile([EH, GL], f32, tag="aggT")
            nc.tensor.matmul(ce_ps[:, :], lhsT=emb_sb[:, :], rhs=clt_sb[:, :],
                             start=True, stop=True)
            ce_sb = workp.tile([EH, GL], mmdt, tag="ce_sb")
            nc.scalar.copy(out=ce_sb[:, :], in_=ce_ps[:, :])
            zt.append(ce_sb)

            # W2: z2^T[128j] = sum_k W2blk[k,j].T @ zt[k]
            w2_sb = workp.tile([128, 6 * 128], mmdt, tag="w2")
            nc.sync.dma_start(out=w2_sb[:, :], in_=w2_d[:, :])
            b2_sb = workp.tile([1, HID], mmdt, tag="b2")
            nc.sync.dma_start(out=b2_sb[:, :], in_=b2_d[:, :])
            ones_g = workp.tile([1, GL], mmdt, tag="onesg")
            nc.vector.memset(ones_g[:, :], 1.0)
            nk = (HID + EH) // 128
            z2 = []
            for jj in range(2):
                zp = ps_h.tile([128, GL], f32, tag="h")
                for kk in range(nk):
                    nc.tensor.matmul(
                        zp[:, :],
                        lhsT=w2_sb[:, (kk * 2 + jj) * 128:(kk * 2 + jj + 1) * 128],
                        rhs=zt[kk][:, :], start=(kk == 0), stop=False)
                nc.tensor.matmul(zp[:, :], lhsT=b2_sb[:, jj * 128:(jj + 1) * 128],
                                 rhs=ones_g[:, :], start=False, stop=True)
                zr_sb = workp.tile([128, GL], f32, tag="zr_sb")
                nc.scalar.activation(zr_sb[:, :], zp[:, :], AF.Relu,
                                     scale=1.0 - NEG)
                z_sb = workp.tile([128, GL], mmdt, tag=f"z2sb{jj}")
                nc.vector.scalar_tensor_tensor(
                    z_sb[:, :], in0=zp[:, :], scalar=NEG, in1=zr_sb[:, :],
                    op0=mybir.AluOpType.mult, op1=mybir.AluOpType.add)
                z2.append(z_sb)

            w3_sb = workp.tile([128, 2], mmdt, tag="w3")
            nc.sync.dma_start(out=w3_sb[:, :], in_=w3_d[:, :])
            b3_sb = workp.tile([1, 1], mmdt, tag="b3")
            nc.sync.dma_start(out=b3_sb[:, :], in_=b3_d[:, :])
            op = ps_h.tile([1, GL], f32, tag="h")
            for jj in range(2):
                nc.tensor.matmul(op[:, :], lhsT=w3_sb[:, jj:jj + 1],
                                 rhs=z2[jj][:, :], start=(jj == 0), stop=False)
            nc.tensor.matmul(op[:, :], lhsT=b3_sb[:, :], rhs=ones_g[:, :],
                             start=False, stop=True)
            o_sb = workp.tile([1, GL], f32, tag="osb")
            nc.vector.tensor_copy(out=o_sb[:, :], in_=op[:, :])
            nc.sync.dma_start(out=out_d[:, :], in_=o_sb[:, :])

    return nc


# ================================================================= runner
def _run(inputs, cfg=None, trace=False):
    from concourse.bass_utils import run_bass_kernel_spmd
    cfg = dict(CFG if cfg is None else cfg)
    prep = host_prep(inputs, cfg)
    nc = build(prep.static)
    nc.finalize()
    res = run_bass_kernel_spmd(
        nc, prep.in_maps, core_ids=list(range(cfg["NCORES"])), trace=trace)
    G = cfg["G"]
    out = np.zeros((G, 1), np.float32)
    for c, r in enumerate(res.results):
        oc = np.asarray(r["out"], np.float32).reshape(-1)
        for j, g in enumerate(prep.static["asg"][c]):
            out[g, 0] = oc[j]
    return out, res


def kernel(**inputs):
    out, _ = _run(inputs)
    return out


# revision 44
# speedup vs baseline: 1.1725x; 1.0266x over previous
"""GCN discriminator kernel for Trainium2 (8 NeuronCores, SPMD).

Math (matching the reference):
  deg[n]  = sum_{e: dst=n} w_e + 1
  dinv    = 1/sqrt(deg)
  norm_e  = dinv[src]*w_e*dinv[dst];  self-loop n: dinv[n]^2
  agg     = sum over incoming edges of norm_e * x[src]         [N, 128]
  h       = leaky_relu(agg @ W1 + b1)                          [N, 256]
  pooled  = segment_mean(h, batch)                             [64, 256]
  z       = leaky_relu(concat(pooled, emb[cls]) @ W2 + b2)
  out     = z @ W3 + b3                                        [64, 1]

Sharding: graph-aligned.  batch is sorted, so graphs occupy contiguous node
ranges; each core owns 8 whole graphs (and the nodes/edge-destinations that
fall in them).  Pooling is therefore core-local — no collective at all; each
core runs the tiny MLP on its own 8 graphs and the host concatenates the
8x[8,1] results.

The irregular gather x[src] runs as HBM dma_gather (SWDGE descriptors)
spread over all 4 SWDGE queues so descriptor emission on the Q7 core-pairs
overlaps the SDMA drains.  Indices are signed-int16 offsets from row XOFF so
one gather covers all 50k source rows.  Self-loop contributions read
contiguous x rows (dense DMA) and are folded in on the Vector engine during
the PSUM evacuation.  The edge segment-sum runs on the PE as one-hot
(weight-scaled) matmuls; pooling is a count-scaled one-hot matmul
accumulated in PSUM.
"""

import numpy as np
import ml_dtypes
from collections import defaultdict

# ----------------------------------------------------------------- config
CFG = dict(
    N=50000, F=128, HID=256, G=64, NCLS=10,
    NCORES=8,
    XOFF=25000,           # gather base row (signed int16 offsets)
    ST_D=128,             # dsts per supertile (= matmul M)
    K=128,                # edge slots per chunk (matmul contraction)
    GRAN=2,               # supertiles per gather granule
    TAPER=4,              # trailing supertiles run as single-st granules
    NEG=0.2,
    NQ=4,                 # SWDGE queues used round-robin for gathers
    SPLIT=2,              # gathers per granule (parallel Q7 emission)
    NO_GATHER=0,          # debug: memset instead of dma_gather
    GDT="bf16",           # gather dtype: "f32" | "bf16"  (x rows + S weights)
    MMDT="bf16",          # downstream matmul dtype
)


def _np_dt(s):
    return {"f32": np.float32, "bf16": ml_dtypes.bfloat16}[s]


# ================================================================= host prep
class Prep:
    pass


def host_prep(inputs, cfg):
    """Integer/layout preprocessing + normalization weights.

    Returns per-core in_maps plus the static (core-independent) program
    structure.
    """
    N, F, G = cfg["N"], cfg["F"], cfg["G"]
    NC, XOFF = cfg["NCORES"], cfg["XOFF"]
    ST_D, K, GRAN = cfg["ST_D"], cfg["K"], cfg["GRAN"]
    GL = G // NC                                   # graphs per core

    x = np.asarray(inputs["x"], np.float32)
    ei = np.asarray(inputs["edge_index"], np.int64)
    ew = np.asarray(inputs["edge_weight"], np.float32)
    batch = np.asarray(inputs["batch"], np.int64)
    cls = np.asarray(inputs["class_labels"], np.int64)
    W1 = np.asarray(inputs["W1"], np.float32)
    b1 = np.asarray(inputs["b1"], np.float32)
    emb = np.asarray(inputs["emb"], np.float32)
    W2 = np.asarray(inputs["W2"], np.float32)
    b2 = np.asarray(inputs["b2"], np.float32)
    W3 = np.asarray(inputs["W3"], np.float32)
    b3 = np.asarray(inputs["b3"], np.float32)

    HID = W1.shape[1]
    EH = emb.shape[1]

    # --- normalization weights (scalar preprocessing, O(E)) -------------
    row = ei[0]
    col = ei[1]
    deg = np.zeros(N, np.float64)
    np.add.at(deg, col, ew.astype(np.float64))
    deg += 1.0
    dinv = 1.0 / np.sqrt(deg)
    a_w = (dinv[row] * ew.astype(np.float64) * dinv[col]).astype(np.float32)
    a_src = row
    a_dst = col
    selfw = (dinv * dinv).astype(np.float32)       # self-loop weights [N]

    # --- balanced graph->core assignment (LPT bin packing, 8 per core) ----
    gcnt = np.bincount(batch, minlength=G)           # nodes per graph
    gcum = np.concatenate([[0], np.cumsum(gcnt)])    # graph start offsets
    load = np.zeros(NC, np.int64)
    nfill = np.zeros(NC, np.int64)
    asg = [[] for _ in range(NC)]                    # graphs per core
    for g in np.argsort(-gcnt, kind="stable"):
        c = min((c for c in range(NC) if nfill[c] < GL),
                key=lambda c: load[c])
        asg[c].append(int(g))
        load[c] += gcnt[g]
        nfill[c] += 1
    nlists = []                                      # per-core node ids
    core_of_node = np.empty(N, np.int64)
    loc_of_node = np.empty(N, np.int64)
    for c in range(NC):
        nl = np.concatenate(
            [np.arange(gcum[g], gcum[g + 1]) for g in asg[c]])
        nlists.append(nl)
        core_of_node[nl] = c
        loc_of_node[nl] = np.arange(len(nl))
    Dmax = int(load.max())
    NST = -(-Dmax // ST_D)      # supertiles per core

    # --- bucket edges into (core, st) ------------------------------------
    core_of = core_of_node[a_dst]
    dst_loc = loc_of_node[a_dst]
    st_of = dst_loc // ST_D
    j_of = dst_loc % ST_D
    key = core_of * NST + st_of

    nbuckets = NC * NST
    cnt = np.bincount(key, minlength=nbuckets).reshape(NC, NST)
    starts = np.zeros(nbuckets + 1, np.int64)
    np.cumsum(cnt.reshape(-1), out=starts[1:])

    # Per-core x-row permutation: order nodes by first-use bucket so each
    # bucket's "home" sources occupy consecutive rows — their 256B gather
    # reads become sequential (HBM row-buffer hits).  Buckets are then
    # sorted by permuted id, making home reads ascending runs.
    perms = []
    seg_src = []   # per-core permuted signed src offsets, bucket-sorted
    seg_j = []
    seg_w = []
    for c in range(NC):
        mask = core_of == c
        st_c = st_of[mask]
        src_c = a_src[mask]
        j_c = j_of[mask]
        w_c = a_w[mask]
        o0 = np.lexsort((src_c, st_c))
        seq = src_c[o0]
        uniq, first_idx = np.unique(seq, return_index=True)
        order_nodes = uniq[np.argsort(first_idx)]
        perm = np.full(N, -1, np.int64)
        perm[order_nodes] = np.arange(len(order_nodes))
        rest = np.nonzero(perm < 0)[0]
        perm[rest] = len(order_nodes) + np.arange(len(rest))
        p_src = perm[src_c]
        o1 = np.lexsort((p_src, st_c))
        perms.append(perm)
        seg_src.append((p_src[o1] - XOFF).astype(np.int32))
        seg_j.append(j_c[o1].astype(np.int32))
        seg_w.append(w_c[o1])

    # static chunk counts per st: max over cores, >= 1; the +1 guarantees
    # at least one padding slot per bucket so every gather's trailing slot
    # is the non-negative padding index (the ucode drops trailing
    # negative-idx runs)
    CH = np.maximum((-(-(cnt + 1) // K)).max(axis=0), 1)   # [NST]

    # --- static program structure ---------------------------------------
    TAPER = cfg.get("TAPER", 0)
    grans = []
    st = 0
    while st < NST:
        n = 1 if st >= NST - TAPER else min(GRAN, NST - st)
        grans.append((st, n))
        st += n

    SELF = -1                                      # self-loop chunk marker
    gmeta = []
    mm_total = 0
    col_total = 0
    for (st0, nst) in grans:
        sts = list(range(st0, st0 + nst))
        nch = int(sum(CH[st] for st in sts))
        cimap = {}
        mms = []
        ci = 0
        for st in sts:
            for k in range(int(CH[st])):
                cimap[(st, k)] = ci
                mms.append((st, k))
                ci += 1
            mms.append((st, SELF))                 # diag(selfw) @ xself[st]
        gmeta.append(dict(st0=st0, nst=nst, nch=nch,
                          cimap=cimap, mms=mms,
                          mm_off=mm_total, col_off=col_total))
        mm_total += len(mms)
        col_total += nch * (K // 16)
    NMM = mm_total
    NCOLS = col_total

    static = dict(cfg=cfg, NST=NST, CH=CH, grans=grans, gmeta=gmeta,
                  NMM=NMM, NCOLS=NCOLS, HID=HID, EH=EH, GL=GL, asg=asg)

    # --- per-core tensors ------------------------------------------------
    gdt = _np_dt(cfg["GDT"])
    mmdt = _np_dt(cfg["MMDT"])

    counts = np.maximum(gcnt, 1).astype(np.float64)

    w2blk = np.zeros((128, 6 * 128), np.float32)
    for kk in range(3):
        for jj in range(2):
            w2blk[:, (kk * 2 + jj) * 128:(kk * 2 + jj + 1) * 128] = \
                W2[kk * 128:(kk + 1) * 128, jj * 128:(jj + 1) * 128]
    w3m = np.zeros((128, 2), np.float32)
    w3m[:, 0] = W3[0:128, 0]
    w3m[:, 1] = W3[128:256, 0]

    xg = np.ascontiguousarray(x).astype(gdt)         # gather/dense source

    in_maps = []
    for c in range(NC):
        cstart = starts[c * NST]
        srcl_s = seg_src[c]
        j_s = seg_j[c]
        w_s = seg_w[c]
        perm = perms[c]
        # permuted per-core x copy: new row perm[i] holds x[i]
        xgc = np.empty_like(xg)
        xgc[perm] = xg
        gidx = np.zeros((NCOLS, 16), np.int16)
        smat = np.zeros((128, NMM * ST_D), gdt)
        for gi, gm in enumerate(gmeta):
            nch = gm["nch"]
            slots_src = np.zeros((nch * K,), np.int32)
            for (st_k, ci) in gm["cimap"].items():
                st, k = st_k
                b = c * NST + st
                s0, s1 = starts[b] - cstart, starts[b + 1] - cstart
                seg = srcl_s[s0:s1]
                part = seg[k * K:(k + 1) * K]
                slots_src[ci * K: ci * K + len(part)] = part
            # gather ucode drops a trailing negative-idx run; gathers are
            # bucket-aligned and padding slots are 0 (>= 0), so this only
            # triggers if a bucket's final chunk is exactly full of
            # all-sub-XOFF sources — vanishingly unlikely with
            # ascending-src bucket order.
            ce = 0
            for st in range(gm["st0"], gm["st0"] + gm["nst"]):
                ce += int(CH[st])
                assert slots_src[ce * K - 1] >= 0, "trailing negative idx"
            c0 = gm["col_off"]
            gidx[c0:c0 + nch * (K // 16), :] = slots_src.reshape(-1, 16)
            # S matrices
            for mi, (st, k) in enumerate(gm["mms"]):
                mm = gm["mm_off"] + mi
                if k < 0:
                    # self-loop diag chunk: agg += diag(selfw) @ xself[st]
                    nl = nlists[c]
                    d0 = st * ST_D
                    nvalid = min(ST_D, max(0, len(nl) - d0))
                    rows = np.arange(nvalid)
                    smat[rows, mm * ST_D + rows] = \
                        selfw[nl[d0:d0 + nvalid]].astype(gdt)
                    continue
                b = c * NST + st
                s0, s1 = starts[b] - cstart, starts[b + 1] - cstart
                jj = j_s[s0:s1][k * K:(k + 1) * K]
                ww = w_s[s0:s1][k * K:(k + 1) * K]
                rows = np.arange(len(jj))
                smat[rows, mm * ST_D + jj] = ww.astype(gdt)
        gidx2 = np.tile(gidx.T, (8, 1))              # [128, NCOLS]

        nl = nlists[c]
        gslot = {g: j for j, g in enumerate(asg[c])}

        # pooling matrix [128, NST*GL], scaled by 1/count (mean pooling)
        pmat = np.zeros((128, NST * GL), np.float64)
        for stn in range(NST):
            d0 = stn * ST_D
            nv = min(ST_D, max(0, len(nl) - d0))
            for p in range(nv):
                g = batch[nl[d0 + p]]
                pmat[p, stn * GL + gslot[g]] = 1.0 / counts[g]

        # self-loop x rows: the core's local nodes in local order, padded
        xself = np.zeros((NST * ST_D, F), gdt)
        xself[:len(nl)] = xg[nl]

        # class embeddings for this core's graphs: clt [NCLS, GL]
        clt = np.zeros((cfg["NCLS"], GL), mmdt)
        clt[cls[asg[c]], np.arange(GL)] = 1.0

        m = dict(
            xg=xgc, xself=xself,
            gidx=np.ascontiguousarray(gidx2),
            smat=smat,
            pmat=pmat.astype(mmdt),
            w1=W1.astype(mmdt),
            w2blk=w2blk.astype(mmdt),
            w3=w3m.astype(mmdt),
            b1=b1.reshape(1, HID).astype(mmdt),
            b2=b2.reshape(1, HID).astype(mmdt),
            b3=b3.reshape(1, 1).astype(mmdt),
            embh=emb.astype(mmdt),
            clt=clt,
        )
        in_maps.append(m)

    prep = Prep()
    prep.static = static
    prep.in_maps = in_maps
    return prep


# ================================================================= builder
def build(static):
    import concourse.bass as bass
    from concourse import bacc, tile
    import concourse.mybir as mybir

    cfg = static["cfg"]
    N, F = cfg["N"], cfg["F"]
    XOFF = cfg["XOFF"]
    ST_D, K = cfg["ST_D"], cfg["K"]
    NST, CH, gmeta = static["NST"], static["CH"], static["gmeta"]
    NMM, NCOLS = static["NMM"], static["NCOLS"]
    HID, EH, GL = static["HID"], static["EH"], static["GL"]
    NCLS = cfg["NCLS"]
    NEG = cfg["NEG"]
    NQ = cfg["NQ"]

    gdt = {"f32": mybir.dt.float32, "bf16": mybir.dt.bfloat16}[cfg["GDT"]]
    mmdt = {"f32": mybir.dt.float32, "bf16": mybir.dt.bfloat16}[cfg["MMDT"]]
    f32 = mybir.dt.float32
    AF = mybir.ActivationFunctionType

    nc = bacc.Bacc(None, target_bir_lowering=False, debug=True,
                   num_swdge_queues=NQ)

    xg_d = nc.declare_dram_parameter("xg", [N, F], gdt, isOutput=False)
    xself_d = nc.declare_dram_parameter("xself", [NST * ST_D, F], gdt, isOutput=False)
    gidx_d = nc.declare_dram_parameter("gidx", [128, NCOLS], mybir.dt.int16, isOutput=False)
    smat_d = nc.declare_dram_parameter("smat", [128, NMM * ST_D], gdt, isOutput=False)
    pmat_d = nc.declare_dram_parameter("pmat", [128, NST * GL], mmdt, isOutput=False)
    w1_d = nc.declare_dram_parameter("w1", [F, HID], mmdt, isOutput=False)
    w2_d = nc.declare_dram_parameter("w2blk", [128, 6 * 128], mmdt, isOutput=False)
    w3_d = nc.declare_dram_parameter("w3", [128, 2], mmdt, isOutput=False)
    b1_d = nc.declare_dram_parameter("b1", [1, HID], mmdt, isOutput=False)
    b2_d = nc.declare_dram_parameter("b2", [1, HID], mmdt, isOutput=False)
    b3_d = nc.declare_dram_parameter("b3", [1, 1], mmdt, isOutput=False)
    emb_d = nc.declare_dram_parameter("embh", [NCLS, EH], mmdt, isOutput=False)
    clt_d = nc.declare_dram_parameter("clt", [NCLS, GL], mmdt, isOutput=False)
    out_d = nc.declare_dram_parameter("out", [1, GL], f32, isOutput=True)

    iden_np = np.eye(128, dtype=_np_dt(cfg["MMDT"]))
    iden_d = nc.inline_tensor(iden_np, name="iden")

    with tile.TileContext(nc) as tc:
        with (
            tc.tile_pool(name="const", bufs=1) as constp,
            tc.tile_pool(name="gix", bufs=6) as gixp,
            tc.tile_pool(name="gat", bufs=9) as gatp,
            tc.tile_pool(name="xsf", bufs=6) as xsfp,
            tc.tile_pool(name="smp", bufs=3) as smp,
            tc.tile_pool(name="work", bufs=3) as workp,
            tc.tile_pool(name="ps_agg", bufs=2, space="PSUM") as ps_agg,
            tc.tile_pool(name="ps_t", bufs=2, space="PSUM") as ps_t,
            tc.tile_pool(name="ps_h", bufs=2, space="PSUM") as ps_h,
            tc.tile_pool(name="ps_pool", bufs=1, space="PSUM") as ps_pool,
        ):
            # ---- persistent SBUF loads
            pmat_sb = constp.tile([128, NST * GL], mmdt)
            nc.sync.dma_start(out=pmat_sb[:, :], in_=pmat_d[:, :])
            w1_sb = constp.tile([F, HID], mmdt)
            nc.sync.dma_start(out=w1_sb[:, :], in_=w1_d[:, :])
            iden_sb = constp.tile([128, 128], mmdt)
            nc.sync.dma_start(out=iden_sb[:, :], in_=iden_d[:, :])
            b1_sb = constp.tile([1, HID], mmdt)
            nc.sync.dma_start(out=b1_sb[:, :], in_=b1_d[:, :])
            ones_sb = constp.tile([1, 128], mmdt)
            nc.vector.memset(ones_sb[:, :], 1.0)

            pooled_ps = ps_pool.tile([GL, HID], f32)

            # ---------------- main loop over granules
            qn = 0
            for gi, gm in enumerate(gmeta):
                st0, nst = gm["st0"], gm["nst"]
                nch = gm["nch"]
                c0 = gm["col_off"]
                gx = gixp.tile([128, nch * (K // 16)], mybir.dt.int16, tag="gx")
                nc.scalar.dma_start(
                    out=gx[:, :],
                    in_=gidx_d[:, c0:c0 + nch * (K // 16)])
                # one gather AND one destination tile per supertile
                # (bucket-aligned so the trailing-negative-idx drop can't
                # hit real slots; separate tiles so the paired gathers have
                # no same-tile write dependency), spread over SWDGE queues
                # for parallel Q7 emission
                gtl = []
                ca = 0
                for sti, st in enumerate(range(st0, st0 + nst)):
                    nck = int(CH[st])
                    gtt = gatp.tile([128, nck, F], gdt, tag=f"gt{sti}",
                                    name="gtt")
                    if cfg.get("NO_GATHER"):
                        nc.vector.memset(gtt[:, :, :], 0.125)
                    else:
                        nc.gpsimd.dma_gather(
                            gtt[:, :, :], xg_d[XOFF:, :],
                            gx[:, ca * (K // 16):(ca + nck) * (K // 16)],
                            num_idxs=nck * K,
                            num_idxs_reg=nck * K,
                            elem_size=F, single_packet=bool(cfg.get("SP1", 0)),
                            queue_num=qn % NQ)
                        qn += 1
                    gtl.append(gtt)
                    ca += nck

                # dense self-loop rows for these supertiles
                xsf = xsfp.tile([128, nst, F], gdt, tag="xsf")
                for sti in range(nst):
                    nc.scalar.dma_start(
                        out=xsf[:, sti, :],
                        in_=xself_d[(st0 + sti) * ST_D:(st0 + sti + 1) * ST_D, :])

                nmm_g = len(gm["mms"])
                sm_sb = smp.tile([128, nmm_g * ST_D], gdt, tag="sm")
                m0 = gm["mm_off"]
                nc.sync.dma_start(
                    out=sm_sb[:, :],
                    in_=smat_d[:, m0 * ST_D:(m0 + nmm_g) * ST_D])

                by_st = defaultdict(list)
                for mi, (st, k) in enumerate(gm["mms"]):
                    by_st[st].append((mi, k))

                for sti, st in enumerate(range(st0, st0 + nst)):
                    # S-matmuls with gathered rows STATIONARY: the PSUM
                    # tile accumulates agg^T = gt.T @ S directly in the
                    # [feature, dst] layout the W1 matmul wants as lhsT —
                    # no transpose step needed.
                    agg = ps_agg.tile([128, F], f32, tag="agg")
                    lst = by_st[st]
                    for i, (mi, k) in enumerate(lst):
                        if k < 0:
                            lhsT = xsf[:, sti, :]    # self-loop diag chunk
                        else:
                            lhsT = gtl[sti][:, k, :]
                        nc.tensor.matmul(
                            agg[:, :],
                            lhsT=lhsT,
                            rhs=sm_sb[:, mi * ST_D:(mi + 1) * ST_D],
                            start=(i == 0), stop=(i == len(lst) - 1))
                    aggT_sb = workp.tile([128, 128], mmdt, tag="aggT_sb")
                    nc.vector.tensor_copy(out=aggT_sb[:, :], in_=agg[:, :])
                    # W1 + b1
                    h_ps = ps_h.tile([128, HID], f32, tag="h")
                    nc.tensor.matmul(h_ps[:, :], lhsT=aggT_sb[:, :],
                                     rhs=w1_sb[:, :], start=True, stop=False)
                    nc.tensor.matmul(h_ps[:, :], lhsT=ones_sb[:, 0:128],
                                     rhs=b1_sb[:, :], start=False, stop=True)
                    # leaky relu -> sbuf
                    hr_sb = workp.tile([128, HID], f32, tag="hr_sb")
                    nc.scalar.activation(hr_sb[:, :], h_ps[:, :], AF.Relu,
                                         scale=1.0 - NEG)
                    h_sb = workp.tile([128, HID], mmdt, tag="h_sb")
                    nc.vector.scalar_tensor_tensor(
                        h_sb[:, :], in0=h_ps[:, :], scalar=NEG,
                        in1=hr_sb[:, :], op0=mybir.AluOpType.mult,
                        op1=mybir.AluOpType.add)
                    # mean-pool accumulate (pmat carries 1/count)
                    nc.tensor.matmul(
                        pooled_ps[:, :],
                        lhsT=pmat_sb[:, st * GL:(st + 1) * GL],
                        rhs=h_sb[:, :],
                        start=(st == 0), stop=(st == NST - 1),
                        skip_group_check=True)

            # ---------------- tail: core-local MLP on GL graphs
            pm_sb = workp.tile([GL, HID], mmdt, tag="pm")
            nc.vector.tensor_copy(out=pm_sb[:, :], in_=pooled_ps[:, :])

            # transpose pooled -> z^T rows [128, GL] halves
            zt = []
            for jj in range(HID // 128):
                tp = ps_t.tile([128, GL], mmdt, tag="aggT")
                nc.tensor.transpose(tp[:, :], pm_sb[:, jj * 128:(jj + 1) * 128],
                                    iden_sb[0:GL, 0:GL])
                t_sb = workp.tile([128, GL], mmdt, tag=f"zt{jj}")
                nc.scalar.copy(out=t_sb[:, :], in_=tp[:, :])
                zt.append(t_sb)
            # class-embedding^T [EH, GL]
            emb_sb = workp.tile([NCLS, EH], mmdt, tag="emb")
            nc.sync.dma_start(out=emb_sb[:, :], in_=emb_d[:, :])
            clt_sb = workp.tile([NCLS, GL], mmdt, tag="clt")
            nc.sync.dma_start(out=clt_sb[:, :], in_=clt_d[:, :])
            ce_ps = ps_t.tile([EH, GL], f32, tag="aggT")
            nc.tensor.matmul(ce_ps[:, :], lhsT=emb_sb[:, :], rhs=clt_sb[:, :],
                             start=True, stop=True)
            ce_sb = workp.tile([EH, GL], mmdt, tag="ce_sb")
            nc.scalar.copy(out=ce_sb[:, :], in_=ce_ps[:, :])
            zt.append(ce_sb)

            # W2: z2^T[128j] = sum_k W2blk[k,j].T @ zt[k]
            w2_sb = workp.tile([128, 6 * 128], mmdt, tag="w2")
            nc.sync.dma_start(out=w2_sb[:, :], in_=w2_d[:, :])
            b2_sb = workp.tile([1, HID], mmdt, tag="b2")
            nc.sync.dma_start(out=b2_sb[:, :], in_=b2_d[:, :])
            ones_g = workp.tile([1, GL], mmdt, tag="onesg")
            nc.vector.memset(ones_g[:, :], 1.0)
            nk = (HID + EH) // 128
            z2 = []
            for jj in range(2):
                zp = ps_h.tile([128, GL], f32, tag="h")
                for kk in range(nk):
                    nc.tensor.matmul(
                        zp[:, :],
                        lhsT=w2_sb[:, (kk * 2 + jj) * 128:(kk * 2 + jj + 1) * 128],
                        rhs=zt[kk][:, :], start=(kk == 0), stop=False)
                nc.tensor.matmul(zp[:, :], lhsT=b2_sb[:, jj * 128:(jj + 1) * 128],
                                 rhs=ones_g[:, :], start=False, stop=True)
                zr_sb = workp.tile([128, GL], f32, tag="zr_sb")
                nc.scalar.activation(zr_sb[:, :], zp[:, :], AF.Relu,
                                     scale=1.0 - NEG)
                z_sb = workp.tile([128, GL], mmdt, tag=f"z2sb{jj}")
                nc.vector.scalar_tensor_tensor(
                    z_sb[:, :], in0=zp[:, :], scalar=NEG, in1=zr_sb[:, :],
                    op0=mybir.AluOpType.mult, op1=mybir.AluOpType.add)
                z2.append(z_sb)

            w3_sb = workp.tile([128, 2], mmdt, tag="w3")
            nc.sync.dma_start(out=w3_sb[:, :], in_=w3_d[:, :])
            b3_sb = workp.tile([1, 1], mmdt, tag="b3")
            nc.sync.dma_start(out=b3_sb[:, :], in_=b3_d[:, :])
            op = ps_h.tile([1, GL], f32, tag="h")
            for jj in range(2):
                nc.tensor.matmul(op[:, :], lhsT=w3_sb[:, jj:jj + 1],
                                 rhs=z2[jj][:, :], start=(jj == 0), stop=False)
            nc.tensor.matmul(op[:, :], lhsT=b3_sb[:, :], rhs=ones_g[:, :],
                             start=False, stop=True)
            o_sb = workp.tile([1, GL], f32, tag="osb")
            nc.vector.tensor_copy(out=o_sb[:, :], in_=op[:, :])
            nc.sync.dma_start(out=out_d[:, :], in_=o_sb[:, :])

    return nc


# ================================================================= runner
def _run(inputs, cfg=None, trace=False):
    from concourse.bass_utils import run_bass_kernel_spmd
    cfg = dict(CFG if cfg is None else cfg)
    prep = host_prep(inputs, cfg)
    nc = build(prep.static)
    nc.finalize()
    res = run_bass_kernel_spmd(
        nc, prep.in_maps, core_ids=list(range(cfg["NCORES"])), trace=trace)
    G = cfg["G"]
    out = np.zeros((G, 1), np.float32)
    for c, r in enumerate(res.results):
        oc = np.asarray(r["out"], np.float32).reshape(-1)
        for j, g in enumerate(prep.static["asg"][c]):
            out[g, 0] = oc[j]
    return out, res


def kernel(**inputs):
    out, _ = _run(inputs)
    return out
